# revision 12
# baseline (speedup 1.0000x reference)
"""Distributed causal multi-head attention for TRN2 (8 NeuronCores).

Sharding: tensor-parallel over heads — core i computes heads {2i, 2i+1}
(128 of the 1024 hidden dims) for the whole (batch, seq) = (4, 2048).
All attention runs in transposed layouts (Q^T/K^T as [dh, t], scores as
[k, q]) so no on-chip transposes are needed except a cheap V^T -> V pass.
A single 8-rank AllToAll re-shards from head-parallel to token-parallel
before the output projection: core i ends up with the full 1024-dim
attention output for tokens [i*1024, (i+1)*1024) of the flattened
(8192,) token axis and computes that slice of out = attn @ W_o^T.

Matmuls run as float32r (single-pass fp32 on the PE, ~4x plain fp32).
"""

import sys

sys.path.insert(0, "/opt/trn_rl_repo")

import numpy as np

import concourse.bass as bass
import concourse.tile as tile
from concourse import bacc, mybir
from concourse.bass_utils import run_bass_kernel_spmd
from concourse.masks import make_identity

F32 = mybir.dt.float32
F32R = mybir.dt.float32r
BF16 = mybir.dt.bfloat16

B, S, D = 4, 2048, 1024
N_HEAD, D_HEAD = 16, 64
T = B * S               # 8192 flattened tokens
N_CORES = 8
HPC = N_HEAD // N_CORES  # heads per core = 2
E = HPC * D_HEAD         # 128 local attn dims per core
TW = 512                 # token window for QKV phase
QM = 512                 # query macro-tile for attention
KT = 128                 # key tile
TOK = T // N_CORES       # 1024 tokens owned per core after A2A
SCALE = 1.0 / 8.0        # 1/sqrt(64)
NEG = -1e9

TRACE = False
LAST_EXEC_NS = None
_CACHED_NC = None


def _build():
    nc = bacc.Bacc("TRN2", target_bir_lowering=False, debug=False,
                   num_devices=N_CORES)
    xT = nc.dram_tensor("xT", [D, T], F32R, kind="ExternalInput").ap()
    wqkvT = nc.dram_tensor("wqkvT", [D, 3 * E], F32R, kind="ExternalInput").ap()
    woT = nc.dram_tensor("woT", [D, D], F32R, kind="ExternalInput").ap()
    out = nc.dram_tensor("out", [TOK, D], F32, kind="ExternalOutput").ap()
    cc_in = nc.dram_tensor("cc_in", [N_CORES, E, TOK], F32R).ap()
    cc_out = nc.dram_tensor("cc_out", [N_CORES, E, TOK], F32R).ap()
    rg = [list(range(N_CORES))]

    n_dt = D // 128          # 8 d-tiles (contraction for projections)
    n_tw = T // TW           # 16 token windows
    n_kt = T // KT           # 64 key tiles total

    with tile.TileContext(nc) as tc:
        with (
            tc.tile_pool(name="persist", bufs=1) as pp,
            tc.tile_pool(name="weights", bufs=1) as wp,
        ):
            # ---- persistent SBUF ----
            qt = pp.tile([E, T], BF16, tag="qt")          # Q^T [128, 8192]
            kt_sb = pp.tile([E, T], BF16, tag="kt")       # K^T [128, 8192]
            vb = pp.tile([128, n_kt, 2 * (D_HEAD + 1)], BF16, tag="vb")
            wk_sb = wp.tile([128, n_dt, 3 * E], F32R, tag="wk")
            for dt in range(n_dt):
                nc.sync.dma_start(out=wk_sb[:, dt, :],
                                  in_=wqkvT[dt * 128:(dt + 1) * 128, :])

            # masks for the 4 diagonal k-tile offsets + identity + ones
            masks = []
            for j in range(4):
                m = wp.tile([KT, QM], F32, tag=f"mask{j}")
                nc.gpsimd.memset(m, 0.0)
                nc.gpsimd.affine_select(
                    out=m, in_=m, compare_op=mybir.AluOpType.is_ge,
                    fill=NEG, base=-(j * KT),
                    pattern=[[1, QM]], channel_multiplier=-1,
                )
                masks.append(m)
            ident_f = wp.tile([128, 128], F32, tag="idf")
            make_identity(nc, ident_f)
            ident = wp.tile([128, 128], BF16, tag="idr")
            nc.vector.tensor_copy(ident, ident_f)
            ones_f = wp.tile([128, n_kt], F32, tag="ones")
            nc.vector.memset(ones_f, 1.0)
            ones_row = wp.tile([1, D_HEAD], F32R, tag="ones_r")
            nc.vector.tensor_copy(ones_row, ones_f[0:1, 0:D_HEAD])
            wo_sb = wp.tile([128, n_dt, D], F32R, tag="wo")
            for dt in range(n_dt):
                nc.sync.dma_start(out=wo_sb[:, dt, :],
                                  in_=woT[dt * 128:(dt + 1) * 128, :])

            # ---- phase A: QKV projections (all in transposed layout) ----
            with (
                nc.named_scope("qkv"),
                tc.tile_pool(name="psA", bufs=3, space="PSUM") as psA,
                tc.tile_pool(name="spA", bufs=2) as sp,
                tc.tile_pool(name="scrA", bufs=2) as scr,
            ):
                # ones columns (col 64 and 129 of every k-tile) — written
                # first so attention tiles don't wait on the whole QKV phase
                for h in range(HPC):
                    nc.vector.tensor_copy(
                        vb[:, :, h * (D_HEAD + 1) + D_HEAD:
                               h * (D_HEAD + 1) + D_HEAD + 1],
                        ones_f[:, :, None])
                for tw in range(n_tw):
                    xw = sp.tile([128, n_dt, TW], F32R, tag="xw")
                    for dt in range(n_dt):
                        nc.sync.dma_start(
                            out=xw[:, dt, :],
                            in_=xT[dt * 128:(dt + 1) * 128,
                                   tw * TW:(tw + 1) * TW])
                    for which, dst in ((0, qt), (1, kt_sb)):
                        ps = psA.tile([128, TW], F32, tag="ps_qkv")
                        for dt in range(n_dt):
                            nc.tensor.matmul(
                                ps[0:E, :],
                                wk_sb[:, dt, which * E:(which + 1) * E],
                                xw[:, dt, :],
                                start=(dt == 0), stop=(dt == n_dt - 1))
                        nc.vector.tensor_copy(
                            dst[:, tw * TW:(tw + 1) * TW], ps[0:E, :])
                    # V^T then transpose into natural layout with ones column
                    ps = psA.tile([128, TW], F32, tag="ps_qkv")
                    for dt in range(n_dt):
                        nc.tensor.matmul(
                            ps[0:E, :], wk_sb[:, dt, 2 * E:3 * E],
                            xw[:, dt, :],
                            start=(dt == 0), stop=(dt == n_dt - 1))
                    vt_sb = scr.tile([E, TW], BF16, tag="vt")
                    nc.vector.tensor_copy(vt_sb, ps[0:E, :])
                    for sub in range(TW // 128):   # 4 k-tiles per window
                        k_idx = tw * (TW // 128) + sub
                        pst = psA.tile([128, 128], BF16, tag="ps_vt")
                        nc.tensor.transpose(
                            pst[:, 0:E], vt_sb[:, sub * 128:(sub + 1) * 128],
                            ident)
                        nc.vector.tensor_copy(
                            vb[:, k_idx, :].rearrange(
                                "p (h c) -> p h c", c=D_HEAD + 1)[:, :, 0:D_HEAD],
                            pst[:, 0:E].rearrange(
                                "p (h c) -> p h c", c=D_HEAD))

            # ---- phase B: causal attention, transposed flash-style ----
            with (
                nc.named_scope("attn"),
                tc.tile_pool(name="psS", bufs=4, space="PSUM") as psS,
                tc.tile_pool(name="psO", bufs=2, space="PSUM") as psO,
                tc.tile_pool(name="psBc", bufs=2, space="PSUM") as psBc,
                tc.tile_pool(name="scrBp", bufs=4) as scr,
                tc.tile_pool(name="scrB2", bufs=2) as scr2,
            ):
                pending = [None]

                def flush_norm():
                    if pending[0] is None:
                        return
                    ps_o_p, rec_p, hp_p, shard_p, qc_p = pending[0]
                    pending[0] = None
                    ps_b = psBc.tile([D_HEAD, QM], F32, tag="ps_b")
                    nc.tensor.matmul(
                        ps_b[:, :], ones_row, rec_p, start=True, stop=True)
                    num_sb = scr2.tile([D_HEAD, QM], F32R, tag="num")
                    nc.vector.tensor_copy(num_sb, ps_o_p[0:D_HEAD, :])
                    attn_sb = scr2.tile([D_HEAD, QM], F32R, tag="attn")
                    nc.vector.tensor_mul(attn_sb, num_sb, ps_b)
                    nc.sync.dma_start(
                        out=cc_in[shard_p, hp_p:hp_p + D_HEAD,
                                  qc_p:qc_p + QM],
                        in_=attn_sb)

                for b in range(B):
                    for h in range(HPC):
                        hp = h * D_HEAD
                        for qm in range(S // QM):
                            q0 = b * S + qm * QM
                            n_k = 4 * qm + 4
                            ps_o = psO.tile([128, QM], F32, tag="ps_o")
                            pts = [None] * n_k

                            def emit_scores(k, b=b, hp=hp, qm=qm, q0=q0,
                                            pts=pts):
                                k0 = b * S + k * KT
                                ps_s = psS.tile([KT, QM], F32, tag="ps_s")
                                nc.tensor.matmul(
                                    ps_s[:, :],
                                    kt_sb[hp:hp + D_HEAD, k0:k0 + KT],
                                    qt[hp:hp + D_HEAD, q0:q0 + QM],
                                    start=True, stop=True)
                                pt = scr.tile([KT, QM], BF16, tag="pt")
                                if k >= 4 * qm:  # diagonal: causal mask
                                    nc.vector.scalar_tensor_tensor(
                                        out=pt, in0=ps_s, scalar=SCALE,
                                        in1=masks[k - 4 * qm],
                                        op0=mybir.AluOpType.mult,
                                        op1=mybir.AluOpType.add)
                                    nc.scalar.activation(
                                        out=pt, in_=pt,
                                        func=mybir.ActivationFunctionType.Exp)
                                else:
                                    nc.scalar.activation(
                                        out=pt, in_=ps_s,
                                        func=mybir.ActivationFunctionType.Exp,
                                        scale=SCALE)
                                pts[k] = pt

                            LA = 3   # score/exp lookahead so PE never waits
                            for k in range(min(LA, n_k)):
                                emit_scores(k)
                            flush_norm()   # previous macro-tile's epilogue
                            for k in range(n_k):
                                if k + LA < n_k:
                                    emit_scores(k + LA)
                                nc.tensor.matmul(
                                    ps_o[0:D_HEAD + 1, :],
                                    vb[:, b * (S // KT) + k,
                                       h * (D_HEAD + 1):(h + 1) * (D_HEAD + 1)],
                                    pts[k],
                                    start=(k == 0), stop=(k == n_k - 1))
                                pts[k] = None
                            # immediate DVE part: denominator reciprocal
                            den_sb = scr2.tile([1, QM], F32, tag="den")
                            nc.vector.tensor_copy(
                                den_sb, ps_o[D_HEAD:D_HEAD + 1, :])
                            recip_f = scr2.tile([1, QM], F32, tag="recipf")
                            nc.vector.reciprocal_approx_fast(
                                out=recip_f, in_=den_sb)
                            recip_r = scr2.tile([1, QM], F32R, tag="recipr")
                            nc.vector.tensor_copy(recip_r, recip_f)
                            pending[0] = (ps_o, recip_r, hp,
                                          b * 2 + qm // 2, (qm % 2) * QM)
                flush_norm()

            # ---- phase C: A2A reshard + output projection ----
            with nc.named_scope("a2a"):
                nc.gpsimd.collective_compute(
                    "AllToAll", mybir.AluOpType.bypass,
                    ins=[cc_in.opt()], outs=[cc_out.opt()], replica_groups=rg)

            with (
                nc.named_scope("oproj"),
                tc.tile_pool(name="psC", bufs=3, space="PSUM") as psC,
                tc.tile_pool(name="spC", bufs=2) as sp,
                tc.tile_pool(name="scrC", bufs=2) as scr,
            ):
                for tt in range(TOK // 128):
                    a_sb = sp.tile([128, n_dt, 128], F32R, tag="a")
                    for dt in range(n_dt):
                        nc.sync.dma_start(
                            out=a_sb[:, dt, :],
                            in_=cc_out[dt, :, tt * 128:(tt + 1) * 128])
                    for et in range(D // 512):
                        ps = psC.tile([128, 512], F32, tag="ps_c")
                        for dt in range(n_dt):
                            nc.tensor.matmul(
                                ps[:, :], a_sb[:, dt, :],
                                wo_sb[:, dt, et * 512:(et + 1) * 512],
                                start=(dt == 0), stop=(dt == n_dt - 1))
                        o_sb = scr.tile([128, 512], F32, tag="o")
                        nc.vector.tensor_copy(o_sb, ps)
                        nc.sync.dma_start(
                            out=out[tt * 128:(tt + 1) * 128,
                                    et * 512:(et + 1) * 512],
                            in_=o_sb)
    nc.compile()
    return nc


def kernel(x, W_qkv, W_o):
    global _CACHED_NC, LAST_EXEC_NS
    if _CACHED_NC is None:
        _CACHED_NC = _build()
    nc = _CACHED_NC

    x = np.ascontiguousarray(x, dtype=np.float32)
    xT = np.ascontiguousarray(x.reshape(T, D).T)          # (1024, 8192)
    woT = np.ascontiguousarray(W_o.astype(np.float32).T)  # (1024, 1024)
    in_maps = []
    for i in range(N_CORES):
        rows = []
        for blk in range(3):                               # Q, K, V blocks
            rows.append(W_qkv[blk * D + i * E: blk * D + (i + 1) * E, :])
        wqkvT = np.ascontiguousarray(
            np.concatenate(rows, axis=0).astype(np.float32).T)  # (1024, 384)
        in_maps.append({"xT": xT, "wqkvT": wqkvT, "woT": woT})

    res = run_bass_kernel_spmd(nc, in_maps, core_ids=list(range(N_CORES)),
                               trace=TRACE)
    LAST_EXEC_NS = res.exec_time_ns
    kernel.LAST_RES = res
    full = np.empty((T, D), dtype=np.float32)
    for i in range(N_CORES):
        full[i * TOK:(i + 1) * TOK, :] = res.results[i]["out"]
    return full.reshape(B, S, D)


# revision 13
# speedup vs baseline: 1.2384x; 1.2384x over previous
"""Distributed causal multi-head attention for TRN2 (8 NeuronCores).

Sharding: tensor-parallel over heads — core i computes heads {2i, 2i+1}
(128 of the 1024 hidden dims) for the whole (batch, seq) = (4, 2048).
All attention runs in transposed layouts (Q^T/K^T as [dh, t], scores as
[k, q]) so no on-chip transposes are needed except a cheap V^T -> V pass.
A single 8-rank AllToAll re-shards from head-parallel to token-parallel
before the output projection: core i ends up with the full 1024-dim
attention output for tokens [i*1024, (i+1)*1024) of the flattened
(8192,) token axis and computes that slice of out = attn @ W_o^T.

Matmuls run as float32r (single-pass fp32 on the PE, ~4x plain fp32).
"""

import sys

sys.path.insert(0, "/opt/trn_rl_repo")

import numpy as np
import ml_dtypes

import concourse.bass as bass
import concourse.tile as tile
from concourse import bacc, mybir
from concourse.bass_utils import run_bass_kernel_spmd
from concourse.masks import make_identity

F32 = mybir.dt.float32
F32R = mybir.dt.float32r
BF16 = mybir.dt.bfloat16

B, S, D = 4, 2048, 1024
N_HEAD, D_HEAD = 16, 64
T = B * S               # 8192 flattened tokens
N_CORES = 8
HPC = N_HEAD // N_CORES  # heads per core = 2
E = HPC * D_HEAD         # 128 local attn dims per core
TW = 512                 # token window for QKV phase
QM = 512                 # query macro-tile for attention
KT = 128                 # key tile
TOK = T // N_CORES       # 1024 tokens owned per core after A2A
SCALE = 1.0 / 8.0        # 1/sqrt(64)
NEG = -1e9

TRACE = False
LAST_EXEC_NS = None
_CACHED_NC = None


def _build():
    nc = bacc.Bacc("TRN2", target_bir_lowering=False, debug=False,
                   num_devices=N_CORES)
    xT = nc.dram_tensor("xT", [D, T], F32R, kind="ExternalInput").ap()
    wqkvT = nc.dram_tensor("wqkvT", [D, 3 * E], F32R, kind="ExternalInput").ap()
    woT = nc.dram_tensor("woT", [D, D], BF16, kind="ExternalInput").ap()
    out = nc.dram_tensor("out", [TOK, D], F32, kind="ExternalOutput").ap()
    cc_in1 = nc.dram_tensor("cc_in1", [N_CORES, E, TOK // 2], BF16).ap()
    cc_out1 = nc.dram_tensor("cc_out1", [N_CORES, E, TOK // 2], BF16).ap()
    cc_in2 = nc.dram_tensor("cc_in2", [N_CORES, E, TOK // 2], BF16).ap()
    cc_out2 = nc.dram_tensor("cc_out2", [N_CORES, E, TOK // 2], BF16).ap()
    rg = [list(range(N_CORES))]

    n_dt = D // 128          # 8 d-tiles (contraction for projections)
    n_tw = T // TW           # 16 token windows
    n_kt = T // KT           # 64 key tiles total

    with tile.TileContext(nc) as tc:
        with (
            tc.tile_pool(name="persist", bufs=1) as pp,
            tc.tile_pool(name="weights", bufs=1) as wp,
        ):
            # ---- persistent SBUF ----
            qt = pp.tile([E, T], BF16, tag="qt")          # Q^T [128, 8192]
            kt_sb = pp.tile([E, T], BF16, tag="kt")       # K^T [128, 8192]
            vb = pp.tile([128, n_kt, 2 * (D_HEAD + 1)], BF16, tag="vb")
            wk_sb = wp.tile([128, n_dt, 3 * E], F32R, tag="wk")
            for dt in range(n_dt):
                nc.sync.dma_start(out=wk_sb[:, dt, :],
                                  in_=wqkvT[dt * 128:(dt + 1) * 128, :])

            # masks for the 4 diagonal k-tile offsets + identity + ones
            masks = []
            for j in range(4):
                m = wp.tile([KT, QM], F32, tag=f"mask{j}")
                nc.gpsimd.memset(m, 0.0)
                nc.gpsimd.affine_select(
                    out=m, in_=m, compare_op=mybir.AluOpType.is_ge,
                    fill=NEG, base=-(j * KT),
                    pattern=[[1, QM]], channel_multiplier=-1,
                )
                masks.append(m)
            ident_f = wp.tile([128, 128], F32, tag="idf")
            make_identity(nc, ident_f)
            ident = wp.tile([128, 128], BF16, tag="idr")
            nc.vector.tensor_copy(ident, ident_f)
            ones_f = wp.tile([128, n_kt], F32, tag="ones")
            nc.vector.memset(ones_f, 1.0)
            ones_row = wp.tile([1, D_HEAD], F32R, tag="ones_r")
            nc.vector.tensor_copy(ones_row, ones_f[0:1, 0:D_HEAD])
            wo_sb = wp.tile([128, n_dt, D], BF16, tag="wo")
            for dt in range(n_dt):
                nc.sync.dma_start(out=wo_sb[:, dt, :],
                                  in_=woT[dt * 128:(dt + 1) * 128, :])

            # ---- phase A: QKV projections (all in transposed layout) ----
            with (
                nc.named_scope("qkv"),
                tc.tile_pool(name="psA", bufs=3, space="PSUM") as psA,
                tc.tile_pool(name="spA", bufs=2) as sp,
                tc.tile_pool(name="scrA", bufs=2) as scr,
            ):
                # ones columns (col 64 and 129 of every k-tile) — written
                # first so attention tiles don't wait on the whole QKV phase
                for h in range(HPC):
                    nc.vector.tensor_copy(
                        vb[:, :, h * (D_HEAD + 1) + D_HEAD:
                               h * (D_HEAD + 1) + D_HEAD + 1],
                        ones_f[:, :, None])
                for tw in range(n_tw):
                    xw = sp.tile([128, n_dt, TW], F32R, tag="xw")
                    for dt in range(n_dt):
                        nc.sync.dma_start(
                            out=xw[:, dt, :],
                            in_=xT[dt * 128:(dt + 1) * 128,
                                   tw * TW:(tw + 1) * TW])
                    for which, dst in ((0, qt), (1, kt_sb)):
                        ps = psA.tile([128, TW], F32, tag="ps_qkv")
                        for dt in range(n_dt):
                            nc.tensor.matmul(
                                ps[0:E, :],
                                wk_sb[:, dt, which * E:(which + 1) * E],
                                xw[:, dt, :],
                                start=(dt == 0), stop=(dt == n_dt - 1))
                        nc.vector.tensor_copy(
                            dst[:, tw * TW:(tw + 1) * TW], ps[0:E, :])
                    # V^T then transpose into natural layout with ones column
                    ps = psA.tile([128, TW], F32, tag="ps_qkv")
                    for dt in range(n_dt):
                        nc.tensor.matmul(
                            ps[0:E, :], wk_sb[:, dt, 2 * E:3 * E],
                            xw[:, dt, :],
                            start=(dt == 0), stop=(dt == n_dt - 1))
                    vt_sb = scr.tile([E, TW], BF16, tag="vt")
                    nc.vector.tensor_copy(vt_sb, ps[0:E, :])
                    for sub in range(TW // 128):   # 4 k-tiles per window
                        k_idx = tw * (TW // 128) + sub
                        pst = psA.tile([128, 128], BF16, tag="ps_vt")
                        nc.tensor.transpose(
                            pst[:, 0:E], vt_sb[:, sub * 128:(sub + 1) * 128],
                            ident)
                        nc.vector.tensor_copy(
                            vb[:, k_idx, :].rearrange(
                                "p (h c) -> p h c", c=D_HEAD + 1)[:, :, 0:D_HEAD],
                            pst[:, 0:E].rearrange(
                                "p (h c) -> p h c", c=D_HEAD))

            # ---- phase B: causal attention, transposed flash-style ----
            # qm order (0,2),(1,3): first halves of every owner's token
            # range finish first, so the A2A is split in two and the first
            # collective overlaps the second half of attention.
            with (
                nc.named_scope("attn"),
                tc.tile_pool(name="psS", bufs=4, space="PSUM") as psS,
                tc.tile_pool(name="psO", bufs=2, space="PSUM") as psO,
                tc.tile_pool(name="psBc", bufs=2, space="PSUM") as psBc,
                tc.tile_pool(name="scrBp", bufs=4) as scr,
                tc.tile_pool(name="scrB2", bufs=2) as scr2,
            ):
                pending = [None]

                def flush_norm():
                    if pending[0] is None:
                        return
                    ps_o_p, rec_p, hp_p, buf_p, shard_p = pending[0]
                    pending[0] = None
                    ps_b = psBc.tile([D_HEAD, QM], F32, tag="ps_b")
                    nc.tensor.matmul(
                        ps_b[:, :], ones_row, rec_p, start=True, stop=True)
                    num_sb = scr2.tile([D_HEAD, QM], F32R, tag="num")
                    nc.vector.tensor_copy(num_sb, ps_o_p[0:D_HEAD, :])
                    attn_sb = scr2.tile([D_HEAD, QM], BF16, tag="attn")
                    nc.vector.tensor_mul(attn_sb, num_sb, ps_b)
                    nc.sync.dma_start(
                        out=buf_p[shard_p, hp_p:hp_p + D_HEAD, :],
                        in_=attn_sb)

                def attn_unit(b, h, qm, cc_buf):
                    hp = h * D_HEAD
                    q0 = b * S + qm * QM
                    n_k = 4 * qm + 4
                    ps_o = psO.tile([128, QM], F32, tag="ps_o")
                    pts = [None] * n_k

                    def emit_scores(k):
                        k0 = b * S + k * KT
                        ps_s = psS.tile([KT, QM], F32, tag="ps_s")
                        nc.tensor.matmul(
                            ps_s[:, :],
                            kt_sb[hp:hp + D_HEAD, k0:k0 + KT],
                            qt[hp:hp + D_HEAD, q0:q0 + QM],
                            start=True, stop=True)
                        pt = scr.tile([KT, QM], BF16, tag="pt")
                        if k >= 4 * qm:  # diagonal: causal mask
                            nc.vector.scalar_tensor_tensor(
                                out=pt, in0=ps_s, scalar=SCALE,
                                in1=masks[k - 4 * qm],
                                op0=mybir.AluOpType.mult,
                                op1=mybir.AluOpType.add)
                            nc.scalar.activation(
                                out=pt, in_=pt,
                                func=mybir.ActivationFunctionType.Exp)
                        else:
                            nc.scalar.activation(
                                out=pt, in_=ps_s,
                                func=mybir.ActivationFunctionType.Exp,
                                scale=SCALE)
                        pts[k] = pt

                    LA = 3   # score/exp lookahead so PE never waits
                    for k in range(min(LA, n_k)):
                        emit_scores(k)
                    flush_norm()   # previous macro-tile's epilogue
                    for k in range(n_k):
                        if k + LA < n_k:
                            emit_scores(k + LA)
                        nc.tensor.matmul(
                            ps_o[0:D_HEAD + 1, :],
                            vb[:, b * (S // KT) + k,
                               h * (D_HEAD + 1):(h + 1) * (D_HEAD + 1)],
                            pts[k],
                            start=(k == 0), stop=(k == n_k - 1))
                        pts[k] = None
                    # immediate DVE part: denominator reciprocal
                    den_sb = scr2.tile([1, QM], F32, tag="den")
                    nc.vector.tensor_copy(den_sb, ps_o[D_HEAD:D_HEAD + 1, :])
                    recip_f = scr2.tile([1, QM], F32, tag="recipf")
                    nc.vector.reciprocal_approx_fast(
                        out=recip_f, in_=den_sb)
                    recip_r = scr2.tile([1, QM], F32R, tag="recipr")
                    nc.vector.tensor_copy(recip_r, recip_f)
                    shard = b * 2 + (0 if qm in (0, 1) else 1)
                    pending[0] = (ps_o, recip_r, hp, cc_buf, shard)

                for qm in (0, 2):
                    for b in range(B):
                        for h in range(HPC):
                            attn_unit(b, h, qm, cc_in1)
                flush_norm()
                with nc.named_scope("a2a1"):
                    nc.gpsimd.collective_compute(
                        "AllToAll", mybir.AluOpType.bypass,
                        ins=[cc_in1.opt()], outs=[cc_out1.opt()],
                        replica_groups=rg)
                for qm in (1, 3):
                    for b in range(B):
                        for h in range(HPC):
                            attn_unit(b, h, qm, cc_in2)
                flush_norm()
                with nc.named_scope("a2a2"):
                    nc.gpsimd.collective_compute(
                        "AllToAll", mybir.AluOpType.bypass,
                        ins=[cc_in2.opt()], outs=[cc_out2.opt()],
                        replica_groups=rg)

            # ---- phase C: output projection on owned tokens ----
            with (
                nc.named_scope("oproj"),
                tc.tile_pool(name="psC", bufs=3, space="PSUM") as psC,
                tc.tile_pool(name="spC", bufs=3) as sp,
                tc.tile_pool(name="scrC", bufs=3) as scr,
            ):
                for tt in range(TOK // 128):
                    cc_o = cc_out1 if tt < 4 else cc_out2
                    col = (tt % 4) * 128
                    a_sb = sp.tile([128, n_dt, 128], BF16, tag="a")
                    for dt in range(n_dt):
                        nc.sync.dma_start(
                            out=a_sb[:, dt, :],
                            in_=cc_o[dt, :, col:col + 128])
                    for et in range(D // 512):
                        ps = psC.tile([128, 512], F32, tag="ps_c")
                        for dt in range(n_dt):
                            nc.tensor.matmul(
                                ps[:, :], a_sb[:, dt, :],
                                wo_sb[:, dt, et * 512:(et + 1) * 512],
                                start=(dt == 0), stop=(dt == n_dt - 1))
                        o_sb = scr.tile([128, 512], F32, tag="o")
                        nc.vector.tensor_copy(o_sb, ps)
                        nc.sync.dma_start(
                            out=out[tt * 128:(tt + 1) * 128,
                                    et * 512:(et + 1) * 512],
                            in_=o_sb)
    nc.compile()
    return nc


def kernel(x, W_qkv, W_o):
    global _CACHED_NC, LAST_EXEC_NS
    if _CACHED_NC is None:
        _CACHED_NC = _build()
    nc = _CACHED_NC

    x = np.ascontiguousarray(x, dtype=np.float32)
    xT = np.ascontiguousarray(x.reshape(T, D).T)          # (1024, 8192)
    woT = np.ascontiguousarray(W_o.astype(np.float32).T).astype(ml_dtypes.bfloat16)
    in_maps = []
    for i in range(N_CORES):
        rows = []
        for blk in range(3):                               # Q, K, V blocks
            rows.append(W_qkv[blk * D + i * E: blk * D + (i + 1) * E, :])
        wqkvT = np.ascontiguousarray(
            np.concatenate(rows, axis=0).astype(np.float32).T)  # (1024, 384)
        in_maps.append({"xT": xT, "wqkvT": wqkvT, "woT": woT})

    res = run_bass_kernel_spmd(nc, in_maps, core_ids=list(range(N_CORES)),
                               trace=TRACE)
    LAST_EXEC_NS = res.exec_time_ns
    kernel.LAST_RES = res
    full = np.empty((T, D), dtype=np.float32)
    for i in range(N_CORES):
        full[i * TOK:(i + 1) * TOK, :] = res.results[i]["out"]
    return full.reshape(B, S, D)


# revision 14
# speedup vs baseline: 1.5290x; 1.2346x over previous
"""Distributed causal multi-head attention for TRN2 (8 NeuronCores).

Sharding: tensor-parallel over heads — core i computes heads {2i, 2i+1}
(128 of the 1024 hidden dims) for the whole (batch, seq) = (4, 2048).
All attention runs in transposed layouts (Q^T/K^T as [dh, t], scores as
[k, q]) so no on-chip transposes are needed except a cheap V^T -> V pass.
A single 8-rank AllToAll re-shards from head-parallel to token-parallel
before the output projection: core i ends up with the full 1024-dim
attention output for tokens [i*1024, (i+1)*1024) of the flattened
(8192,) token axis and computes that slice of out = attn @ W_o^T.

Matmuls run as float32r (single-pass fp32 on the PE, ~4x plain fp32).
"""

import sys

sys.path.insert(0, "/opt/trn_rl_repo")

import numpy as np
import ml_dtypes

import concourse.bass as bass
import concourse.tile as tile
from concourse import bacc, mybir
from concourse.bass_utils import run_bass_kernel_spmd
from concourse.masks import make_identity

F32 = mybir.dt.float32
F32R = mybir.dt.float32r
BF16 = mybir.dt.bfloat16

B, S, D = 4, 2048, 1024
N_HEAD, D_HEAD = 16, 64
T = B * S               # 8192 flattened tokens
N_CORES = 8
HPC = N_HEAD // N_CORES  # heads per core = 2
E = HPC * D_HEAD         # 128 local attn dims per core
TW = 512                 # token window for QKV phase
QM = 512                 # query macro-tile for attention
KT = 128                 # key tile
TOK = T // N_CORES       # 1024 tokens owned per core after A2A
SCALE = 1.0 / 8.0        # 1/sqrt(64)
NEG = -1e9

TRACE = False
LAST_EXEC_NS = None
_CACHED_NC = None


def _build():
    nc = bacc.Bacc("TRN2", target_bir_lowering=False, debug=False,
                   num_devices=N_CORES)
    xT = nc.dram_tensor("xT", [D, T], F32R, kind="ExternalInput").ap()
    wqkvT = nc.dram_tensor("wqkvT", [D, 3 * E], F32R, kind="ExternalInput").ap()
    woT = nc.dram_tensor("woT", [D, D], BF16, kind="ExternalInput").ap()
    out = nc.dram_tensor("out", [TOK, D], F32, kind="ExternalOutput").ap()
    cc_in1 = nc.dram_tensor("cc_in1", [N_CORES, E, TOK // 2], BF16).ap()
    cc_out1 = nc.dram_tensor("cc_out1", [N_CORES, E, TOK // 2], BF16).ap()
    cc_in2 = nc.dram_tensor("cc_in2", [N_CORES, E, TOK // 2], BF16).ap()
    cc_out2 = nc.dram_tensor("cc_out2", [N_CORES, E, TOK // 2], BF16).ap()
    rg = [list(range(N_CORES))]

    n_dt = D // 128          # 8 d-tiles (contraction for projections)
    n_tw = T // TW           # 16 token windows
    n_kt = T // KT           # 64 key tiles total

    with tile.TileContext(nc) as tc:
        with (
            tc.tile_pool(name="persist", bufs=1) as pp,
            tc.tile_pool(name="weights", bufs=1) as wp,
        ):
            # ---- persistent SBUF ----
            qt = pp.tile([128, HPC, T], BF16, tag="qt")   # zero-padded per head
            kt_sb = pp.tile([E, T], BF16, tag="kt")       # K^T [128, 8192]
            vb = pp.tile([128, n_kt, 2 * (D_HEAD + 1)], BF16, tag="vb")
            wk_sb = wp.tile([128, n_dt, 3 * E], F32R, tag="wk")
            for dt in range(n_dt):
                nc.sync.dma_start(out=wk_sb[:, dt, :],
                                  in_=wqkvT[dt * 128:(dt + 1) * 128, :])

            # masks for the 4 diagonal k-tile offsets + identity + ones
            masks = []
            for j in range(4):
                m = wp.tile([KT, QM], F32, tag=f"mask{j}")
                nc.gpsimd.memset(m, 0.0)
                nc.gpsimd.affine_select(
                    out=m, in_=m, compare_op=mybir.AluOpType.is_ge,
                    fill=NEG, base=-(j * KT),
                    pattern=[[1, QM]], channel_multiplier=-1,
                )
                masks.append(m)
            ident_f = wp.tile([128, 128], F32, tag="idf")
            make_identity(nc, ident_f)
            ident = wp.tile([128, 128], BF16, tag="idr")
            nc.vector.tensor_copy(ident, ident_f)
            ones_f = wp.tile([128, n_kt], F32, tag="ones")
            nc.vector.memset(ones_f, 1.0)
            ones_row = wp.tile([1, D_HEAD], F32R, tag="ones_r")
            nc.vector.tensor_copy(ones_row, ones_f[0:1, 0:D_HEAD])
            wo_sb = wp.tile([128, n_dt, D], BF16, tag="wo")
            for dt in range(n_dt):
                nc.sync.dma_start(out=wo_sb[:, dt, :],
                                  in_=woT[dt * 128:(dt + 1) * 128, :])

            # ---- phase A: QKV projections (all in transposed layout) ----
            with (
                nc.named_scope("qkv"),
                tc.tile_pool(name="psA", bufs=3, space="PSUM") as psA,
                tc.tile_pool(name="spA", bufs=2) as sp,
                tc.tile_pool(name="scrA", bufs=2) as scr,
            ):
                # zero the pad rows of qt so per-head scores can contract
                # over all 128 partitions (full-array matmuls stay HAM-warm)
                nc.vector.memset(qt[D_HEAD:128, 0, :], 0.0)
                nc.vector.memset(qt[0:D_HEAD, 1, :], 0.0)
                # ones columns (col 64 and 129 of every k-tile) — written
                # first so attention tiles don't wait on the whole QKV phase
                for h in range(HPC):
                    nc.vector.tensor_copy(
                        vb[:, :, h * (D_HEAD + 1) + D_HEAD:
                               h * (D_HEAD + 1) + D_HEAD + 1],
                        ones_f[:, :, None])
                for tw in range(n_tw):
                    xw = sp.tile([128, n_dt, TW], F32R, tag="xw")
                    for dt in range(n_dt):
                        nc.sync.dma_start(
                            out=xw[:, dt, :],
                            in_=xT[dt * 128:(dt + 1) * 128,
                                   tw * TW:(tw + 1) * TW])
                    for which in (0, 1):
                        ps = psA.tile([128, TW], F32, tag="ps_qkv")
                        for dt in range(n_dt):
                            nc.tensor.matmul(
                                ps[0:E, :],
                                wk_sb[:, dt, which * E:(which + 1) * E],
                                xw[:, dt, :],
                                start=(dt == 0), stop=(dt == n_dt - 1))
                        if which == 1:
                            nc.vector.tensor_copy(
                                kt_sb[:, tw * TW:(tw + 1) * TW], ps[0:E, :])
                        else:
                            for h in range(HPC):
                                nc.vector.tensor_copy(
                                    qt[h * D_HEAD:(h + 1) * D_HEAD, h,
                                       tw * TW:(tw + 1) * TW],
                                    ps[h * D_HEAD:(h + 1) * D_HEAD, :])
                    # V^T then transpose into natural layout with ones column
                    ps = psA.tile([128, TW], F32, tag="ps_qkv")
                    for dt in range(n_dt):
                        nc.tensor.matmul(
                            ps[0:E, :], wk_sb[:, dt, 2 * E:3 * E],
                            xw[:, dt, :],
                            start=(dt == 0), stop=(dt == n_dt - 1))
                    vt_sb = scr.tile([E, TW], BF16, tag="vt")
                    nc.vector.tensor_copy(vt_sb, ps[0:E, :])
                    for sub in range(TW // 128):   # 4 k-tiles per window
                        k_idx = tw * (TW // 128) + sub
                        pst = psA.tile([128, 128], BF16, tag="ps_vt")
                        nc.tensor.transpose(
                            pst[:, 0:E], vt_sb[:, sub * 128:(sub + 1) * 128],
                            ident)
                        nc.vector.tensor_copy(
                            vb[:, k_idx, :].rearrange(
                                "p (h c) -> p h c", c=D_HEAD + 1)[:, :, 0:D_HEAD],
                            pst[:, 0:E].rearrange(
                                "p (h c) -> p h c", c=D_HEAD))

            # ---- phase B: causal attention, transposed flash-style ----
            # qm order (0,2),(1,3): first halves of every owner's token
            # range finish first, so the A2A is split in two and the first
            # collective overlaps the second half of attention.
            with (
                nc.named_scope("attn"),
                tc.tile_pool(name="psS", bufs=4, space="PSUM") as psS,
                tc.tile_pool(name="psO", bufs=2, space="PSUM") as psO,
                tc.tile_pool(name="psBc", bufs=2, space="PSUM") as psBc,
                tc.tile_pool(name="scrBp", bufs=4) as scr,
                tc.tile_pool(name="scrB2", bufs=2) as scr2,
            ):
                pending = [None]

                def flush_norm():
                    if pending[0] is None:
                        return
                    ps_o_p, rec_p, hp_p, buf_p, shard_p = pending[0]
                    pending[0] = None
                    ps_b = psBc.tile([D_HEAD, QM], F32, tag="ps_b")
                    nc.tensor.matmul(
                        ps_b[:, :], ones_row, rec_p, start=True, stop=True)
                    num_sb = scr2.tile([D_HEAD, QM], F32R, tag="num")
                    nc.vector.tensor_copy(num_sb, ps_o_p[0:D_HEAD, :])
                    attn_sb = scr2.tile([D_HEAD, QM], BF16, tag="attn")
                    nc.vector.tensor_mul(attn_sb, num_sb, ps_b)
                    nc.sync.dma_start(
                        out=buf_p[shard_p, hp_p:hp_p + D_HEAD, :],
                        in_=attn_sb)

                def attn_unit(b, h, qm, cc_buf):
                    hp = h * D_HEAD
                    q0 = b * S + qm * QM
                    n_k = 4 * qm + 4
                    ps_o = psO.tile([128, QM], F32, tag="ps_o")
                    pts = [None] * n_k

                    def emit_scores(k):
                        k0 = b * S + k * KT
                        ps_s = psS.tile([KT, QM], F32, tag="ps_s")
                        nc.tensor.matmul(
                            ps_s[:, :],
                            kt_sb[:, k0:k0 + KT],
                            qt[:, h, q0:q0 + QM],
                            start=True, stop=True)
                        pt = scr.tile([KT, QM], BF16, tag="pt")
                        if k >= 4 * qm:  # diagonal: causal mask
                            nc.vector.scalar_tensor_tensor(
                                out=pt, in0=ps_s, scalar=SCALE,
                                in1=masks[k - 4 * qm],
                                op0=mybir.AluOpType.mult,
                                op1=mybir.AluOpType.add)
                            nc.scalar.activation(
                                out=pt, in_=pt,
                                func=mybir.ActivationFunctionType.Exp)
                        else:
                            nc.scalar.activation(
                                out=pt, in_=ps_s,
                                func=mybir.ActivationFunctionType.Exp,
                                scale=SCALE)
                        pts[k] = pt

                    LA = 3   # score/exp lookahead so PE never waits
                    for k in range(min(LA, n_k)):
                        emit_scores(k)
                    flush_norm()   # previous macro-tile's epilogue
                    for k in range(n_k):
                        if k + LA < n_k:
                            emit_scores(k + LA)
                        nc.tensor.matmul(
                            ps_o[0:D_HEAD + 1, :],
                            vb[:, b * (S // KT) + k,
                               h * (D_HEAD + 1):(h + 1) * (D_HEAD + 1)],
                            pts[k],
                            start=(k == 0), stop=(k == n_k - 1))
                        pts[k] = None
                    # immediate DVE part: denominator reciprocal
                    den_sb = scr2.tile([1, QM], F32, tag="den")
                    nc.vector.tensor_copy(den_sb, ps_o[D_HEAD:D_HEAD + 1, :])
                    recip_f = scr2.tile([1, QM], F32, tag="recipf")
                    nc.vector.reciprocal_approx_fast(
                        out=recip_f, in_=den_sb)
                    recip_r = scr2.tile([1, QM], F32R, tag="recipr")
                    nc.vector.tensor_copy(recip_r, recip_f)
                    shard = b * 2 + (0 if qm in (0, 1) else 1)
                    pending[0] = (ps_o, recip_r, hp, cc_buf, shard)

                for qm in (0, 2):
                    for b in range(B):
                        for h in range(HPC):
                            attn_unit(b, h, qm, cc_in1)
                flush_norm()
                with nc.named_scope("a2a1"):
                    nc.gpsimd.collective_compute(
                        "AllToAll", mybir.AluOpType.bypass,
                        ins=[cc_in1.opt()], outs=[cc_out1.opt()],
                        replica_groups=rg)
                for qm in (1, 3):
                    for b in range(B):
                        for h in range(HPC):
                            attn_unit(b, h, qm, cc_in2)
                flush_norm()
                with nc.named_scope("a2a2"):
                    nc.gpsimd.collective_compute(
                        "AllToAll", mybir.AluOpType.bypass,
                        ins=[cc_in2.opt()], outs=[cc_out2.opt()],
                        replica_groups=rg)

            # ---- phase C: output projection on owned tokens ----
            with (
                nc.named_scope("oproj"),
                tc.tile_pool(name="psC", bufs=3, space="PSUM") as psC,
                tc.tile_pool(name="spC", bufs=3) as sp,
                tc.tile_pool(name="scrC", bufs=3) as scr,
            ):
                for tt in range(TOK // 128):
                    cc_o = cc_out1 if tt < 4 else cc_out2
                    col = (tt % 4) * 128
                    a_sb = sp.tile([128, n_dt, 128], BF16, tag="a")
                    for dt in range(n_dt):
                        nc.sync.dma_start(
                            out=a_sb[:, dt, :],
                            in_=cc_o[dt, :, col:col + 128])
                    for et in range(D // 512):
                        ps = psC.tile([128, 512], F32, tag="ps_c")
                        for dt in range(n_dt):
                            nc.tensor.matmul(
                                ps[:, :], a_sb[:, dt, :],
                                wo_sb[:, dt, et * 512:(et + 1) * 512],
                                start=(dt == 0), stop=(dt == n_dt - 1))
                        o_sb = scr.tile([128, 512], F32, tag="o")
                        nc.vector.tensor_copy(o_sb, ps)
                        nc.sync.dma_start(
                            out=out[tt * 128:(tt + 1) * 128,
                                    et * 512:(et + 1) * 512],
                            in_=o_sb)
    nc.compile()
    return nc


def kernel(x, W_qkv, W_o):
    global _CACHED_NC, LAST_EXEC_NS
    if _CACHED_NC is None:
        _CACHED_NC = _build()
    nc = _CACHED_NC

    x = np.ascontiguousarray(x, dtype=np.float32)
    xT = np.ascontiguousarray(x.reshape(T, D).T)          # (1024, 8192)
    woT = np.ascontiguousarray(W_o.astype(np.float32).T).astype(ml_dtypes.bfloat16)
    in_maps = []
    for i in range(N_CORES):
        rows = []
        for blk in range(3):                               # Q, K, V blocks
            rows.append(W_qkv[blk * D + i * E: blk * D + (i + 1) * E, :])
        wqkvT = np.ascontiguousarray(
            np.concatenate(rows, axis=0).astype(np.float32).T)  # (1024, 384)
        in_maps.append({"xT": xT, "wqkvT": wqkvT, "woT": woT})

    res = run_bass_kernel_spmd(nc, in_maps, core_ids=list(range(N_CORES)),
                               trace=TRACE)
    LAST_EXEC_NS = res.exec_time_ns
    kernel.LAST_RES = res
    full = np.empty((T, D), dtype=np.float32)
    for i in range(N_CORES):
        full[i * TOK:(i + 1) * TOK, :] = res.results[i]["out"]
    return full.reshape(B, S, D)


# revision 15
# speedup vs baseline: 1.5390x; 1.0065x over previous
"""Distributed causal multi-head attention for TRN2 (8 NeuronCores).

Sharding: tensor-parallel over heads — core i computes heads {2i, 2i+1}
(128 of the 1024 hidden dims) for the whole (batch, seq) = (4, 2048).
All attention runs in transposed layouts (Q^T/K^T as [dh, t], scores as
[k, q]) so no on-chip transposes are needed except a cheap V^T -> V pass.
A single 8-rank AllToAll re-shards from head-parallel to token-parallel
before the output projection: core i ends up with the full 1024-dim
attention output for tokens [i*1024, (i+1)*1024) of the flattened
(8192,) token axis and computes that slice of out = attn @ W_o^T.

Matmuls run as float32r (single-pass fp32 on the PE, ~4x plain fp32).
"""

import sys

sys.path.insert(0, "/opt/trn_rl_repo")

import numpy as np
import ml_dtypes

import concourse.bass as bass
import concourse.tile as tile
from concourse import bacc, mybir
from concourse.bass_utils import run_bass_kernel_spmd
from concourse.masks import make_identity

F32 = mybir.dt.float32
F32R = mybir.dt.float32r
BF16 = mybir.dt.bfloat16

B, S, D = 4, 2048, 1024
N_HEAD, D_HEAD = 16, 64
T = B * S               # 8192 flattened tokens
N_CORES = 8
HPC = N_HEAD // N_CORES  # heads per core = 2
E = HPC * D_HEAD         # 128 local attn dims per core
TW = 512                 # token window for QKV phase
QM = 512                 # query macro-tile for attention
KT = 128                 # key tile
TOK = T // N_CORES       # 1024 tokens owned per core after A2A
SCALE = 1.0 / 8.0        # 1/sqrt(64)
NEG = -1e9

TRACE = False
LAST_EXEC_NS = None
_CACHED_NC = None


def _build():
    nc = bacc.Bacc("TRN2", target_bir_lowering=False, debug=False,
                   num_devices=N_CORES)
    xT = nc.dram_tensor("xT", [D, T], F32R, kind="ExternalInput").ap()
    wqkvT = nc.dram_tensor("wqkvT", [D, 3 * E], F32R, kind="ExternalInput").ap()
    woT = nc.dram_tensor("woT", [D, D], BF16, kind="ExternalInput").ap()
    out = nc.dram_tensor("out", [TOK, D], F32, kind="ExternalOutput").ap()
    cc_in1 = nc.dram_tensor("cc_in1", [N_CORES, E, TOK // 2], BF16).ap()
    cc_out1 = nc.dram_tensor("cc_out1", [N_CORES, E, TOK // 2], BF16).ap()
    cc_in2 = nc.dram_tensor("cc_in2", [N_CORES, E, TOK // 2], BF16).ap()
    cc_out2 = nc.dram_tensor("cc_out2", [N_CORES, E, TOK // 2], BF16).ap()
    rg = [list(range(N_CORES))]

    n_dt = D // 128          # 8 d-tiles (contraction for projections)
    n_tw = T // TW           # 16 token windows
    n_kt = T // KT           # 64 key tiles total

    with tile.TileContext(nc) as tc:
        with (
            tc.tile_pool(name="persist", bufs=1) as pp,
            tc.tile_pool(name="weights", bufs=1) as wp,
        ):
            # ---- persistent SBUF ----
            qt = pp.tile([128, HPC, T], BF16, tag="qt")   # zero-padded per head
            kt_sb = pp.tile([E, T], BF16, tag="kt")       # K^T [128, 8192]
            vb = pp.tile([128, n_kt, 2 * (D_HEAD + 1)], BF16, tag="vb")
            wk_sb = wp.tile([128, n_dt, 3 * E], F32R, tag="wk")
            for dt in range(n_dt):
                nc.sync.dma_start(out=wk_sb[:, dt, :],
                                  in_=wqkvT[dt * 128:(dt + 1) * 128, :])

            # masks for the 4 diagonal k-tile offsets + identity + ones
            masks = []
            for j in range(4):
                m = wp.tile([KT, QM], F32, tag=f"mask{j}")
                nc.gpsimd.memset(m, 0.0)
                nc.gpsimd.affine_select(
                    out=m, in_=m, compare_op=mybir.AluOpType.is_ge,
                    fill=NEG, base=-(j * KT),
                    pattern=[[1, QM]], channel_multiplier=-1,
                )
                masks.append(m)
            ident_f = wp.tile([128, 128], F32, tag="idf")
            make_identity(nc, ident_f)
            ident = wp.tile([128, 128], BF16, tag="idr")
            nc.vector.tensor_copy(ident, ident_f)
            ones_f = wp.tile([128, n_kt], F32, tag="ones")
            nc.vector.memset(ones_f, 1.0)
            ones_row = wp.tile([1, D_HEAD], F32R, tag="ones_r")
            nc.vector.tensor_copy(ones_row, ones_f[0:1, 0:D_HEAD])
            wo_sb = wp.tile([128, n_dt, D], BF16, tag="wo")
            for dt in range(n_dt):
                nc.sync.dma_start(out=wo_sb[:, dt, :],
                                  in_=woT[dt * 128:(dt + 1) * 128, :])

            # ---- phase A: QKV projections (all in transposed layout) ----
            with (
                nc.named_scope("qkv"),
                tc.tile_pool(name="psA", bufs=3, space="PSUM") as psA,
                tc.tile_pool(name="spA", bufs=3) as sp,
                tc.tile_pool(name="scrA", bufs=2) as scr,
            ):
                # zero the pad rows of qt so per-head scores can contract
                # over all 128 partitions (full-array matmuls stay HAM-warm)
                nc.vector.memset(qt[D_HEAD:128, 0, :], 0.0)
                nc.vector.memset(qt[0:D_HEAD, 1, :], 0.0)
                # ones columns (col 64 and 129 of every k-tile) — written
                # first so attention tiles don't wait on the whole QKV phase
                for h in range(HPC):
                    nc.vector.tensor_copy(
                        vb[:, :, h * (D_HEAD + 1) + D_HEAD:
                               h * (D_HEAD + 1) + D_HEAD + 1],
                        ones_f[:, :, None])
                for tw in range(n_tw):
                    xw = sp.tile([128, n_dt, TW], F32R, tag="xw")
                    for dt in range(n_dt):
                        nc.sync.dma_start(
                            out=xw[:, dt, :],
                            in_=xT[dt * 128:(dt + 1) * 128,
                                   tw * TW:(tw + 1) * TW])
                    for which in (0, 1):
                        ps = psA.tile([128, TW], F32, tag="ps_qkv")
                        for dt in range(n_dt):
                            nc.tensor.matmul(
                                ps[0:E, :],
                                wk_sb[:, dt, which * E:(which + 1) * E],
                                xw[:, dt, :],
                                start=(dt == 0), stop=(dt == n_dt - 1))
                        if which == 1:
                            nc.vector.tensor_copy(
                                kt_sb[:, tw * TW:(tw + 1) * TW], ps[0:E, :])
                        else:
                            for h in range(HPC):
                                nc.vector.tensor_copy(
                                    qt[h * D_HEAD:(h + 1) * D_HEAD, h,
                                       tw * TW:(tw + 1) * TW],
                                    ps[h * D_HEAD:(h + 1) * D_HEAD, :])
                    # V^T then transpose into natural layout with ones column
                    ps = psA.tile([128, TW], F32, tag="ps_qkv")
                    for dt in range(n_dt):
                        nc.tensor.matmul(
                            ps[0:E, :], wk_sb[:, dt, 2 * E:3 * E],
                            xw[:, dt, :],
                            start=(dt == 0), stop=(dt == n_dt - 1))
                    vt_sb = scr.tile([E, TW], BF16, tag="vt")
                    nc.vector.tensor_copy(vt_sb, ps[0:E, :])
                    for sub in range(TW // 128):   # 4 k-tiles per window
                        k_idx = tw * (TW // 128) + sub
                        pst = psA.tile([128, 128], BF16, tag="ps_vt")
                        nc.tensor.transpose(
                            pst[:, 0:E], vt_sb[:, sub * 128:(sub + 1) * 128],
                            ident)
                        nc.vector.tensor_copy(
                            vb[:, k_idx, :].rearrange(
                                "p (h c) -> p h c", c=D_HEAD + 1)[:, :, 0:D_HEAD],
                            pst[:, 0:E].rearrange(
                                "p (h c) -> p h c", c=D_HEAD))

            # ---- phase B: causal attention, transposed flash-style ----
            # qm order (0,2),(1,3): first halves of every owner's token
            # range finish first, so the A2A is split in two and the first
            # collective overlaps the second half of attention.
            with (
                nc.named_scope("attn"),
                tc.tile_pool(name="psS", bufs=4, space="PSUM") as psS,
                tc.tile_pool(name="psO", bufs=3, space="PSUM") as psO,
                tc.tile_pool(name="psBc", bufs=1, space="PSUM") as psBc,
                tc.tile_pool(name="scrBp", bufs=4) as scr,
                tc.tile_pool(name="scrB2", bufs=2) as scr2,
            ):
                pending = [None]

                def flush_norm():
                    if pending[0] is None:
                        return
                    ps_o_p, rec_p, hp_p, buf_p, shard_p = pending[0]
                    pending[0] = None
                    ps_b = psBc.tile([D_HEAD, QM], F32, tag="ps_b")
                    nc.tensor.matmul(
                        ps_b[:, :], ones_row, rec_p, start=True, stop=True)
                    num_sb = scr2.tile([D_HEAD, QM], F32R, tag="num")
                    nc.vector.tensor_copy(num_sb, ps_o_p[0:D_HEAD, :])
                    attn_sb = scr2.tile([D_HEAD, QM], BF16, tag="attn")
                    nc.vector.tensor_mul(attn_sb, num_sb, ps_b)
                    nc.sync.dma_start(
                        out=buf_p[shard_p, hp_p:hp_p + D_HEAD, :],
                        in_=attn_sb)

                def attn_unit(b, h, qm, cc_buf):
                    hp = h * D_HEAD
                    q0 = b * S + qm * QM
                    n_k = 4 * qm + 4
                    ps_o = psO.tile([128, QM], F32, tag="ps_o")
                    pts = [None] * n_k

                    def emit_scores(k):
                        k0 = b * S + k * KT
                        ps_s = psS.tile([KT, QM], F32, tag="ps_s")
                        nc.tensor.matmul(
                            ps_s[:, :],
                            kt_sb[:, k0:k0 + KT],
                            qt[:, h, q0:q0 + QM],
                            start=True, stop=True)
                        pt = scr.tile([KT, QM], BF16, tag="pt")
                        if k >= 4 * qm:  # diagonal: causal mask
                            nc.vector.scalar_tensor_tensor(
                                out=pt, in0=ps_s, scalar=SCALE,
                                in1=masks[k - 4 * qm],
                                op0=mybir.AluOpType.mult,
                                op1=mybir.AluOpType.add)
                            nc.scalar.activation(
                                out=pt, in_=pt,
                                func=mybir.ActivationFunctionType.Exp)
                        else:
                            nc.scalar.activation(
                                out=pt, in_=ps_s,
                                func=mybir.ActivationFunctionType.Exp,
                                scale=SCALE)
                        pts[k] = pt

                    LA = 3   # score/exp lookahead so PE never waits
                    for k in range(min(LA, n_k)):
                        emit_scores(k)
                    flush_norm()   # previous macro-tile's epilogue
                    for k in range(n_k):
                        if k + LA < n_k:
                            emit_scores(k + LA)
                        nc.tensor.matmul(
                            ps_o[0:D_HEAD + 1, :],
                            vb[:, b * (S // KT) + k,
                               h * (D_HEAD + 1):(h + 1) * (D_HEAD + 1)],
                            pts[k],
                            start=(k == 0), stop=(k == n_k - 1))
                        pts[k] = None
                    # immediate DVE part: denominator reciprocal
                    den_sb = scr2.tile([1, QM], F32, tag="den")
                    nc.vector.tensor_copy(den_sb, ps_o[D_HEAD:D_HEAD + 1, :])
                    recip_f = scr2.tile([1, QM], F32, tag="recipf")
                    nc.vector.reciprocal_approx_fast(
                        out=recip_f, in_=den_sb)
                    recip_r = scr2.tile([1, QM], F32R, tag="recipr")
                    nc.vector.tensor_copy(recip_r, recip_f)
                    shard = b * 2 + (0 if qm in (0, 1) else 1)
                    pending[0] = (ps_o, recip_r, hp, cc_buf, shard)

                for qm in (0, 2):
                    for b in range(B):
                        for h in range(HPC):
                            attn_unit(b, h, qm, cc_in1)
                flush_norm()
                with nc.named_scope("a2a1"):
                    nc.gpsimd.collective_compute(
                        "AllToAll", mybir.AluOpType.bypass,
                        ins=[cc_in1.opt()], outs=[cc_out1.opt()],
                        replica_groups=rg)
                for qm in (1, 3):
                    for b in range(B):
                        for h in range(HPC):
                            attn_unit(b, h, qm, cc_in2)
                flush_norm()
                with nc.named_scope("a2a2"):
                    nc.gpsimd.collective_compute(
                        "AllToAll", mybir.AluOpType.bypass,
                        ins=[cc_in2.opt()], outs=[cc_out2.opt()],
                        replica_groups=rg)

            # ---- phase C: output projection on owned tokens ----
            with (
                nc.named_scope("oproj"),
                tc.tile_pool(name="psC", bufs=3, space="PSUM") as psC,
                tc.tile_pool(name="spC", bufs=3) as sp,
                tc.tile_pool(name="scrC", bufs=3) as scr,
            ):
                for tt in range(TOK // 128):
                    cc_o = cc_out1 if tt < 4 else cc_out2
                    col = (tt % 4) * 128
                    a_sb = sp.tile([128, n_dt, 128], BF16, tag="a")
                    for dt in range(n_dt):
                        nc.sync.dma_start(
                            out=a_sb[:, dt, :],
                            in_=cc_o[dt, :, col:col + 128])
                    for et in range(D // 512):
                        ps = psC.tile([128, 512], F32, tag="ps_c")
                        for dt in range(n_dt):
                            nc.tensor.matmul(
                                ps[:, :], a_sb[:, dt, :],
                                wo_sb[:, dt, et * 512:(et + 1) * 512],
                                start=(dt == 0), stop=(dt == n_dt - 1))
                        o_sb = scr.tile([128, 512], F32, tag="o")
                        nc.vector.tensor_copy(o_sb, ps)
                        nc.sync.dma_start(
                            out=out[tt * 128:(tt + 1) * 128,
                                    et * 512:(et + 1) * 512],
                            in_=o_sb)
    nc.compile()
    return nc


def kernel(x, W_qkv, W_o):
    global _CACHED_NC, LAST_EXEC_NS
    if _CACHED_NC is None:
        _CACHED_NC = _build()
    nc = _CACHED_NC

    x = np.ascontiguousarray(x, dtype=np.float32)
    xT = np.ascontiguousarray(x.reshape(T, D).T)          # (1024, 8192)
    woT = np.ascontiguousarray(W_o.astype(np.float32).T).astype(ml_dtypes.bfloat16)
    in_maps = []
    for i in range(N_CORES):
        rows = []
        for blk in range(3):                               # Q, K, V blocks
            rows.append(W_qkv[blk * D + i * E: blk * D + (i + 1) * E, :])
        wqkvT = np.ascontiguousarray(
            np.concatenate(rows, axis=0).astype(np.float32).T)  # (1024, 384)
        in_maps.append({"xT": xT, "wqkvT": wqkvT, "woT": woT})

    res = run_bass_kernel_spmd(nc, in_maps, core_ids=list(range(N_CORES)),
                               trace=TRACE)
    LAST_EXEC_NS = res.exec_time_ns
    kernel.LAST_RES = res
    full = np.empty((T, D), dtype=np.float32)
    for i in range(N_CORES):
        full[i * TOK:(i + 1) * TOK, :] = res.results[i]["out"]
    return full.reshape(B, S, D)


# revision 17
# speedup vs baseline: 1.5434x; 1.0028x over previous
"""Distributed causal multi-head attention for TRN2 (8 NeuronCores).

Sharding: tensor-parallel over heads — core i computes heads {2i, 2i+1}
(128 of the 1024 hidden dims) for the whole (batch, seq) = (4, 2048).
All attention runs in transposed layouts (Q^T/K^T as [dh, t], scores as
[k, q]) so no on-chip transposes are needed except a cheap V^T -> V pass.
A single 8-rank AllToAll re-shards from head-parallel to token-parallel
before the output projection: core i ends up with the full 1024-dim
attention output for tokens [i*1024, (i+1)*1024) of the flattened
(8192,) token axis and computes that slice of out = attn @ W_o^T.

Matmuls run as float32r (single-pass fp32 on the PE, ~4x plain fp32).
"""

import sys

sys.path.insert(0, "/opt/trn_rl_repo")

import numpy as np
import ml_dtypes

import concourse.bass as bass
import concourse.tile as tile
from concourse import bacc, mybir
from concourse.bass_utils import run_bass_kernel_spmd
from concourse.masks import make_identity

F32 = mybir.dt.float32
F32R = mybir.dt.float32r
BF16 = mybir.dt.bfloat16

B, S, D = 4, 2048, 1024
N_HEAD, D_HEAD = 16, 64
T = B * S               # 8192 flattened tokens
N_CORES = 8
HPC = N_HEAD // N_CORES  # heads per core = 2
E = HPC * D_HEAD         # 128 local attn dims per core
TW = 512                 # token window for QKV phase
QM = 512                 # query macro-tile for attention
KT = 128                 # key tile
TOK = T // N_CORES       # 1024 tokens owned per core after A2A
SCALE = 1.0 / 8.0        # 1/sqrt(64)
NEG = -1e9

TRACE = False
LAST_EXEC_NS = None
_CACHED_NC = None


def _build():
    nc = bacc.Bacc("TRN2", target_bir_lowering=False, debug=False,
                   num_devices=N_CORES)
    xT = nc.dram_tensor("xT", [D, T], F32R, kind="ExternalInput").ap()
    wqkvT = nc.dram_tensor("wqkvT", [D, 3 * E], F32R, kind="ExternalInput").ap()
    woT = nc.dram_tensor("woT", [D, D], BF16, kind="ExternalInput").ap()
    out = nc.dram_tensor("out", [TOK, D], F32, kind="ExternalOutput").ap()
    cc_in1 = nc.dram_tensor("cc_in1", [N_CORES, E, TOK // 2], BF16).ap()
    cc_out1 = nc.dram_tensor("cc_out1", [N_CORES, E, TOK // 2], BF16).ap()
    cc_in2 = nc.dram_tensor("cc_in2", [N_CORES, E, TOK // 2], BF16).ap()
    cc_out2 = nc.dram_tensor("cc_out2", [N_CORES, E, TOK // 2], BF16).ap()
    rg = [list(range(N_CORES))]

    n_dt = D // 128          # 8 d-tiles (contraction for projections)
    n_tw = T // TW           # 16 token windows
    n_kt = T // KT           # 64 key tiles total

    with tile.TileContext(nc) as tc:
        with (
            tc.tile_pool(name="persist", bufs=1) as pp,
            tc.tile_pool(name="weights", bufs=1) as wp,
        ):
            # ---- persistent SBUF ----
            qt = pp.tile([128, HPC, T], BF16, tag="qt")   # zero-padded per head
            kt_sb = pp.tile([E, T], BF16, tag="kt")       # K^T [128, 8192]
            vb = pp.tile([128, n_kt, 2 * (D_HEAD + 1)], BF16, tag="vb")
            wk_sb = wp.tile([128, n_dt, 3 * E], F32R, tag="wk")
            for dt in range(n_dt):
                nc.sync.dma_start(out=wk_sb[:, dt, :],
                                  in_=wqkvT[dt * 128:(dt + 1) * 128, :])

            # masks for the 4 diagonal k-tile offsets + identity + ones
            masks = []
            for j in range(4):
                m = wp.tile([KT, QM], F32, tag=f"mask{j}")
                nc.gpsimd.memset(m, 0.0)
                nc.gpsimd.affine_select(
                    out=m, in_=m, compare_op=mybir.AluOpType.is_ge,
                    fill=NEG, base=-(j * KT),
                    pattern=[[1, QM]], channel_multiplier=-1,
                )
                masks.append(m)
            ident_f = wp.tile([128, 128], F32, tag="idf")
            make_identity(nc, ident_f)
            ident = wp.tile([128, 128], BF16, tag="idr")
            nc.vector.tensor_copy(ident, ident_f)
            ones_f = wp.tile([128, n_kt], F32, tag="ones")
            nc.vector.memset(ones_f, 1.0)
            ones_row = wp.tile([1, D_HEAD], F32R, tag="ones_r")
            nc.vector.tensor_copy(ones_row, ones_f[0:1, 0:D_HEAD])
            wo_sb = wp.tile([128, n_dt, D], BF16, tag="wo")
            for dt in range(n_dt):
                nc.sync.dma_start(out=wo_sb[:, dt, :],
                                  in_=woT[dt * 128:(dt + 1) * 128, :])

            # ---- phase A: QKV projections (all in transposed layout) ----
            with (
                nc.named_scope("qkv"),
                tc.tile_pool(name="psA", bufs=3, space="PSUM") as psA,
                tc.tile_pool(name="spA", bufs=3) as sp,
                tc.tile_pool(name="scrA", bufs=2) as scr,
            ):
                # zero the pad rows of qt so per-head scores can contract
                # over all 128 partitions (full-array matmuls stay HAM-warm)
                nc.vector.memset(qt[D_HEAD:128, 0, :], 0.0)
                nc.vector.memset(qt[0:D_HEAD, 1, :], 0.0)
                # ones columns (col 64 and 129 of every k-tile) — written
                # first so attention tiles don't wait on the whole QKV phase
                for h in range(HPC):
                    nc.vector.tensor_copy(
                        vb[:, :, h * (D_HEAD + 1) + D_HEAD:
                               h * (D_HEAD + 1) + D_HEAD + 1],
                        ones_f[:, :, None])
                for tw in range(n_tw):
                    xw = sp.tile([128, n_dt, TW], F32R, tag="xw")
                    for dt in range(n_dt):
                        nc.sync.dma_start(
                            out=xw[:, dt, :],
                            in_=xT[dt * 128:(dt + 1) * 128,
                                   tw * TW:(tw + 1) * TW])
                    for which in (0, 1):
                        ps = psA.tile([128, TW], F32, tag="ps_qkv")
                        for dt in range(n_dt):
                            nc.tensor.matmul(
                                ps[0:E, :],
                                wk_sb[:, dt, which * E:(which + 1) * E],
                                xw[:, dt, :],
                                start=(dt == 0), stop=(dt == n_dt - 1))
                        if which == 1:
                            nc.vector.tensor_copy(
                                kt_sb[:, tw * TW:(tw + 1) * TW], ps[0:E, :])
                        else:
                            for h in range(HPC):
                                nc.vector.tensor_copy(
                                    qt[h * D_HEAD:(h + 1) * D_HEAD, h,
                                       tw * TW:(tw + 1) * TW],
                                    ps[h * D_HEAD:(h + 1) * D_HEAD, :])
                    # V^T then transpose into natural layout with ones column
                    ps = psA.tile([128, TW], F32, tag="ps_qkv")
                    for dt in range(n_dt):
                        nc.tensor.matmul(
                            ps[0:E, :], wk_sb[:, dt, 2 * E:3 * E],
                            xw[:, dt, :],
                            start=(dt == 0), stop=(dt == n_dt - 1))
                    vt_sb = scr.tile([E, TW], BF16, tag="vt")
                    nc.vector.tensor_copy(vt_sb, ps[0:E, :])
                    for sub in range(TW // 128):   # 4 k-tiles per window
                        k_idx = tw * (TW // 128) + sub
                        pst = psA.tile([128, 128], BF16, tag="ps_vt")
                        nc.tensor.transpose(
                            pst[:, 0:E], vt_sb[:, sub * 128:(sub + 1) * 128],
                            ident)
                        nc.vector.tensor_copy(
                            vb[:, k_idx, :].rearrange(
                                "p (h c) -> p h c", c=D_HEAD + 1)[:, :, 0:D_HEAD],
                            pst[:, 0:E].rearrange(
                                "p (h c) -> p h c", c=D_HEAD))

            # ---- phase B: causal attention, transposed flash-style ----
            # qm order (0,2),(1,3): first halves of every owner's token
            # range finish first, so the A2A is split in two and the first
            # collective overlaps the second half of attention.
            with (
                nc.named_scope("attn"),
                tc.tile_pool(name="psS", bufs=4, space="PSUM") as psS,
                tc.tile_pool(name="psO", bufs=3, space="PSUM") as psO,
                tc.tile_pool(name="scrBp", bufs=4) as scr,
                tc.tile_pool(name="scrB2", bufs=2) as scr2,
            ):
                pending = [None]

                def flush_norm():
                    if pending[0] is None:
                        return
                    ps_o_p, rec_p, hp_p, buf_p, shard_p = pending[0]
                    pending[0] = None
                    rb = scr2.tile([D_HEAD, QM], F32, tag="rb")
                    nc.gpsimd.partition_broadcast(rb, rec_p)
                    num_sb = scr2.tile([D_HEAD, QM], F32R, tag="num")
                    nc.vector.tensor_copy(num_sb, ps_o_p[0:D_HEAD, :])
                    attn_sb = scr2.tile([D_HEAD, QM], BF16, tag="attn")
                    nc.vector.tensor_mul(attn_sb, num_sb, rb)
                    nc.sync.dma_start(
                        out=buf_p[shard_p, hp_p:hp_p + D_HEAD, :],
                        in_=attn_sb)

                def attn_unit(b, h, qm, cc_buf):
                    hp = h * D_HEAD
                    q0 = b * S + qm * QM
                    n_k = 4 * qm + 4
                    ps_o = psO.tile([128, QM], F32, tag="ps_o")
                    pts = [None] * n_k

                    def emit_scores(k):
                        k0 = b * S + k * KT
                        ps_s = psS.tile([KT, QM], F32, tag="ps_s")
                        nc.tensor.matmul(
                            ps_s[:, :],
                            kt_sb[:, k0:k0 + KT],
                            qt[:, h, q0:q0 + QM],
                            start=True, stop=True)
                        pt = scr.tile([KT, QM], BF16, tag="pt")
                        if k >= 4 * qm:  # diagonal: causal mask
                            nc.vector.scalar_tensor_tensor(
                                out=pt, in0=ps_s, scalar=SCALE,
                                in1=masks[k - 4 * qm],
                                op0=mybir.AluOpType.mult,
                                op1=mybir.AluOpType.add)
                            nc.scalar.activation(
                                out=pt, in_=pt,
                                func=mybir.ActivationFunctionType.Exp)
                        else:
                            nc.scalar.activation(
                                out=pt, in_=ps_s,
                                func=mybir.ActivationFunctionType.Exp,
                                scale=SCALE)
                        pts[k] = pt

                    LA = 3   # score/exp lookahead so PE never waits
                    for k in range(min(LA, n_k)):
                        emit_scores(k)
                    flush_norm()   # previous macro-tile's epilogue
                    for k in range(n_k):
                        if k + LA < n_k:
                            emit_scores(k + LA)
                        nc.tensor.matmul(
                            ps_o[0:D_HEAD + 1, :],
                            vb[:, b * (S // KT) + k,
                               h * (D_HEAD + 1):(h + 1) * (D_HEAD + 1)],
                            pts[k],
                            start=(k == 0), stop=(k == n_k - 1))
                        pts[k] = None
                    # immediate DVE part: denominator reciprocal
                    den_sb = scr2.tile([1, QM], F32, tag="den")
                    nc.vector.tensor_copy(den_sb, ps_o[D_HEAD:D_HEAD + 1, :])
                    recip_f = scr2.tile([1, QM], F32, tag="recipf")
                    nc.vector.reciprocal_approx_fast(
                        out=recip_f, in_=den_sb)
                    shard = b * 2 + (0 if qm in (0, 1) else 1)
                    pending[0] = (ps_o, recip_f, hp, cc_buf, shard)

                for qm in (0, 2):
                    for b in range(B):
                        for h in range(HPC):
                            attn_unit(b, h, qm, cc_in1)
                flush_norm()
                with nc.named_scope("a2a1"):
                    nc.gpsimd.collective_compute(
                        "AllToAll", mybir.AluOpType.bypass,
                        ins=[cc_in1.opt()], outs=[cc_out1.opt()],
                        replica_groups=rg)
                for qm in (1, 3):
                    for b in range(B):
                        for h in range(HPC):
                            attn_unit(b, h, qm, cc_in2)
                flush_norm()
                with nc.named_scope("a2a2"):
                    nc.gpsimd.collective_compute(
                        "AllToAll", mybir.AluOpType.bypass,
                        ins=[cc_in2.opt()], outs=[cc_out2.opt()],
                        replica_groups=rg)

            # ---- phase C: output projection on owned tokens ----
            with (
                nc.named_scope("oproj"),
                tc.tile_pool(name="psC", bufs=3, space="PSUM") as psC,
                tc.tile_pool(name="spC", bufs=3) as sp,
                tc.tile_pool(name="scrC", bufs=3) as scr,
            ):
                for tt in range(TOK // 128):
                    cc_o = cc_out1 if tt < 4 else cc_out2
                    col = (tt % 4) * 128
                    a_sb = sp.tile([128, n_dt, 128], BF16, tag="a")
                    for dt in range(n_dt):
                        nc.sync.dma_start(
                            out=a_sb[:, dt, :],
                            in_=cc_o[dt, :, col:col + 128])
                    for et in range(D // 512):
                        ps = psC.tile([128, 512], F32, tag="ps_c")
                        for dt in range(n_dt):
                            nc.tensor.matmul(
                                ps[:, :], a_sb[:, dt, :],
                                wo_sb[:, dt, et * 512:(et + 1) * 512],
                                start=(dt == 0), stop=(dt == n_dt - 1))
                        o_sb = scr.tile([128, 512], F32, tag="o")
                        nc.vector.tensor_copy(o_sb, ps)
                        nc.sync.dma_start(
                            out=out[tt * 128:(tt + 1) * 128,
                                    et * 512:(et + 1) * 512],
                            in_=o_sb)
    nc.compile()
    return nc


def kernel(x, W_qkv, W_o):
    global _CACHED_NC, LAST_EXEC_NS
    if _CACHED_NC is None:
        _CACHED_NC = _build()
    nc = _CACHED_NC

    x = np.ascontiguousarray(x, dtype=np.float32)
    xT = np.ascontiguousarray(x.reshape(T, D).T)          # (1024, 8192)
    woT = np.ascontiguousarray(W_o.astype(np.float32).T).astype(ml_dtypes.bfloat16)
    in_maps = []
    for i in range(N_CORES):
        rows = []
        for blk in range(3):                               # Q, K, V blocks
            rows.append(W_qkv[blk * D + i * E: blk * D + (i + 1) * E, :])
        wqkvT = np.ascontiguousarray(
            np.concatenate(rows, axis=0).astype(np.float32).T)  # (1024, 384)
        in_maps.append({"xT": xT, "wqkvT": wqkvT, "woT": woT})

    res = run_bass_kernel_spmd(nc, in_maps, core_ids=list(range(N_CORES)),
                               trace=TRACE)
    LAST_EXEC_NS = res.exec_time_ns
    kernel.LAST_RES = res
    full = np.empty((T, D), dtype=np.float32)
    for i in range(N_CORES):
        full[i * TOK:(i + 1) * TOK, :] = res.results[i]["out"]
    return full.reshape(B, S, D)


# revision 18
# speedup vs baseline: 1.6519x; 1.0704x over previous
"""Distributed causal multi-head attention for TRN2 (8 NeuronCores).

Sharding: tensor-parallel over heads — core i computes heads {2i, 2i+1}
(128 of the 1024 hidden dims) for the whole (batch, seq) = (4, 2048).
All attention runs in transposed layouts (Q^T/K^T as [dh, t], scores as
[k, q]) so no on-chip transposes are needed except a cheap V^T -> V pass.
A single 8-rank AllToAll re-shards from head-parallel to token-parallel
before the output projection: core i ends up with the full 1024-dim
attention output for tokens [i*1024, (i+1)*1024) of the flattened
(8192,) token axis and computes that slice of out = attn @ W_o^T.

Matmuls run as float32r (single-pass fp32 on the PE, ~4x plain fp32).
"""

import sys

sys.path.insert(0, "/opt/trn_rl_repo")

import numpy as np
import ml_dtypes

import concourse.bass as bass
import concourse.tile as tile
from concourse import bacc, mybir
from concourse.bass_utils import run_bass_kernel_spmd
from concourse.masks import make_identity

F32 = mybir.dt.float32
F32R = mybir.dt.float32r
BF16 = mybir.dt.bfloat16

B, S, D = 4, 2048, 1024
N_HEAD, D_HEAD = 16, 64
T = B * S               # 8192 flattened tokens
N_CORES = 8
HPC = N_HEAD // N_CORES  # heads per core = 2
E = HPC * D_HEAD         # 128 local attn dims per core
TW = 512                 # token window for QKV phase
QM = 512                 # query macro-tile for attention
KT = 128                 # key tile
TOK = T // N_CORES       # 1024 tokens owned per core after A2A
SCALE = 1.0 / 8.0        # 1/sqrt(64)
NEG = -1e9

TRACE = False
LAST_EXEC_NS = None
_CACHED_NC = None


def _build():
    nc = bacc.Bacc("TRN2", target_bir_lowering=False, debug=False,
                   num_devices=N_CORES)
    xT = nc.dram_tensor("xT", [D, T], F32R, kind="ExternalInput").ap()
    wqkvT = nc.dram_tensor("wqkvT", [D, 3 * E], F32R, kind="ExternalInput").ap()
    woT = nc.dram_tensor("woT", [D, D], BF16, kind="ExternalInput").ap()
    out = nc.dram_tensor("out", [TOK, D], F32, kind="ExternalOutput").ap()
    cc_in1 = nc.dram_tensor("cc_in1", [N_CORES, E, TOK // 2], BF16).ap()
    cc_out1 = nc.dram_tensor("cc_out1", [N_CORES, E, TOK // 2], BF16).ap()
    cc_in2 = nc.dram_tensor("cc_in2", [N_CORES, E, TOK // 2], BF16).ap()
    cc_out2 = nc.dram_tensor("cc_out2", [N_CORES, E, TOK // 2], BF16).ap()
    rg = [list(range(N_CORES))]

    n_dt = D // 128          # 8 d-tiles (contraction for projections)
    n_tw = T // TW           # 16 token windows
    n_kt = T // KT           # 64 key tiles total

    with tile.TileContext(nc) as tc:
        with (
            tc.tile_pool(name="persist", bufs=1) as pp,
            tc.tile_pool(name="weights", bufs=1) as wp,
        ):
            # ---- persistent SBUF ----
            qt = pp.tile([128, HPC, T], BF16, tag="qt")   # zero-padded per head
            kt_sb = pp.tile([E, T], BF16, tag="kt")       # K^T [128, 8192]
            vb = pp.tile([128, n_kt, 2 * (D_HEAD + 1)], BF16, tag="vb")
            wk_sb = wp.tile([128, n_dt, 3 * E], F32R, tag="wk")
            for dt in range(n_dt):
                nc.sync.dma_start(out=wk_sb[:, dt, :],
                                  in_=wqkvT[dt * 128:(dt + 1) * 128, :])

            # masks for the 4 diagonal k-tile offsets + identity + ones
            masks = []
            for j in range(4):
                m = wp.tile([KT, QM], BF16, tag=f"mask{j}")
                nc.gpsimd.memset(m, 1.0)
                nc.gpsimd.affine_select(
                    out=m, in_=m, compare_op=mybir.AluOpType.is_ge,
                    fill=0.0, base=-(j * KT),
                    pattern=[[1, QM]], channel_multiplier=-1,
                )
                masks.append(m)
            ident_f = wp.tile([128, 128], F32, tag="idf")
            make_identity(nc, ident_f)
            ident = wp.tile([128, 128], BF16, tag="idr")
            nc.vector.tensor_copy(ident, ident_f)
            ones_f = wp.tile([128, n_kt], F32, tag="ones")
            nc.vector.memset(ones_f, 1.0)
            ones_row = wp.tile([1, D_HEAD], F32R, tag="ones_r")
            nc.vector.tensor_copy(ones_row, ones_f[0:1, 0:D_HEAD])
            wo_sb = wp.tile([128, n_dt, D], BF16, tag="wo")
            for dt in range(n_dt):
                nc.sync.dma_start(out=wo_sb[:, dt, :],
                                  in_=woT[dt * 128:(dt + 1) * 128, :])

            # ---- phase A: QKV projections (all in transposed layout) ----
            with (
                nc.named_scope("qkv"),
                tc.tile_pool(name="psA", bufs=3, space="PSUM") as psA,
                tc.tile_pool(name="spA", bufs=3) as sp,
                tc.tile_pool(name="scrA", bufs=2) as scr,
            ):
                # zero the pad rows of qt so per-head scores can contract
                # over all 128 partitions (full-array matmuls stay HAM-warm)
                nc.vector.memset(qt[D_HEAD:128, 0, :], 0.0)
                nc.vector.memset(qt[0:D_HEAD, 1, :], 0.0)
                # ones columns (col 64 and 129 of every k-tile) — written
                # first so attention tiles don't wait on the whole QKV phase
                for h in range(HPC):
                    nc.vector.tensor_copy(
                        vb[:, :, h * (D_HEAD + 1) + D_HEAD:
                               h * (D_HEAD + 1) + D_HEAD + 1],
                        ones_f[:, :, None])
                for tw in range(n_tw):
                    xw = sp.tile([128, n_dt, TW], F32R, tag="xw")
                    for dt in range(n_dt):
                        nc.sync.dma_start(
                            out=xw[:, dt, :],
                            in_=xT[dt * 128:(dt + 1) * 128,
                                   tw * TW:(tw + 1) * TW])
                    for which in (0, 1):
                        ps = psA.tile([128, TW], F32, tag="ps_qkv")
                        for dt in range(n_dt):
                            nc.tensor.matmul(
                                ps[0:E, :],
                                wk_sb[:, dt, which * E:(which + 1) * E],
                                xw[:, dt, :],
                                start=(dt == 0), stop=(dt == n_dt - 1))
                        if which == 1:
                            nc.vector.tensor_copy(
                                kt_sb[:, tw * TW:(tw + 1) * TW], ps[0:E, :])
                        else:
                            for h in range(HPC):
                                nc.vector.tensor_copy(
                                    qt[h * D_HEAD:(h + 1) * D_HEAD, h,
                                       tw * TW:(tw + 1) * TW],
                                    ps[h * D_HEAD:(h + 1) * D_HEAD, :])
                    # V^T then transpose into natural layout with ones column
                    ps = psA.tile([128, TW], F32, tag="ps_qkv")
                    for dt in range(n_dt):
                        nc.tensor.matmul(
                            ps[0:E, :], wk_sb[:, dt, 2 * E:3 * E],
                            xw[:, dt, :],
                            start=(dt == 0), stop=(dt == n_dt - 1))
                    vt_sb = scr.tile([E, TW], BF16, tag="vt")
                    nc.vector.tensor_copy(vt_sb, ps[0:E, :])
                    for sub in range(TW // 128):   # 4 k-tiles per window
                        k_idx = tw * (TW // 128) + sub
                        pst = psA.tile([128, 128], BF16, tag="ps_vt")
                        nc.tensor.transpose(
                            pst[:, 0:E], vt_sb[:, sub * 128:(sub + 1) * 128],
                            ident)
                        nc.vector.tensor_copy(
                            vb[:, k_idx, :].rearrange(
                                "p (h c) -> p h c", c=D_HEAD + 1)[:, :, 0:D_HEAD],
                            pst[:, 0:E].rearrange(
                                "p (h c) -> p h c", c=D_HEAD))

            # ---- phase B: causal attention, transposed flash-style ----
            # qm order (0,2),(1,3): first halves of every owner's token
            # range finish first, so the A2A is split in two and the first
            # collective overlaps the second half of attention.
            with (
                nc.named_scope("attn"),
                tc.tile_pool(name="psS", bufs=4, space="PSUM") as psS,
                tc.tile_pool(name="psO", bufs=3, space="PSUM") as psO,
                tc.tile_pool(name="scrBp", bufs=4) as scr,
                tc.tile_pool(name="scrB2", bufs=2) as scr2,
            ):
                pending = [None]

                def flush_norm():
                    if pending[0] is None:
                        return
                    ps_o_p, rec_p, hp_p, buf_p, shard_p = pending[0]
                    pending[0] = None
                    rb = scr2.tile([D_HEAD, QM], F32, tag="rb")
                    nc.gpsimd.partition_broadcast(rb, rec_p)
                    num_sb = scr2.tile([D_HEAD, QM], F32R, tag="num")
                    nc.vector.tensor_copy(num_sb, ps_o_p[0:D_HEAD, :])
                    attn_sb = scr2.tile([D_HEAD, QM], BF16, tag="attn")
                    nc.vector.tensor_mul(attn_sb, num_sb, rb)
                    nc.sync.dma_start(
                        out=buf_p[shard_p, hp_p:hp_p + D_HEAD, :],
                        in_=attn_sb)

                def attn_unit(b, h, qm, cc_buf):
                    hp = h * D_HEAD
                    q0 = b * S + qm * QM
                    n_k = 4 * qm + 4
                    ps_o = psO.tile([128, QM], F32, tag="ps_o")
                    pts = [None] * n_k

                    def emit_scores(k):
                        k0 = b * S + k * KT
                        ps_s = psS.tile([KT, QM], F32, tag="ps_s")
                        nc.tensor.matmul(
                            ps_s[:, :],
                            kt_sb[:, k0:k0 + KT],
                            qt[:, h, q0:q0 + QM],
                            start=True, stop=True)
                        pt = scr.tile([KT, QM], BF16, tag="pt")
                        if k >= 4 * qm:  # diagonal: exp then zero-mask
                            pe = scr.tile([KT, QM], BF16, tag="pe")
                            nc.scalar.activation(
                                out=pe, in_=ps_s,
                                func=mybir.ActivationFunctionType.Exp,
                                scale=SCALE)
                            nc.vector.tensor_mul(
                                pt, pe, masks[k - 4 * qm])
                        else:
                            nc.scalar.activation(
                                out=pt, in_=ps_s,
                                func=mybir.ActivationFunctionType.Exp,
                                scale=SCALE)
                        pts[k] = pt

                    LA = 3   # score/exp lookahead so PE never waits
                    for k in range(min(LA, n_k)):
                        emit_scores(k)
                    flush_norm()   # previous macro-tile's epilogue
                    for k in range(n_k):
                        if k + LA < n_k:
                            emit_scores(k + LA)
                        nc.tensor.matmul(
                            ps_o[0:D_HEAD + 1, :],
                            vb[:, b * (S // KT) + k,
                               h * (D_HEAD + 1):(h + 1) * (D_HEAD + 1)],
                            pts[k],
                            start=(k == 0), stop=(k == n_k - 1))
                        pts[k] = None
                    # immediate DVE part: denominator reciprocal
                    den_sb = scr2.tile([1, QM], F32, tag="den")
                    nc.vector.tensor_copy(den_sb, ps_o[D_HEAD:D_HEAD + 1, :])
                    recip_f = scr2.tile([1, QM], F32, tag="recipf")
                    nc.vector.reciprocal_approx_fast(
                        out=recip_f, in_=den_sb)
                    shard = b * 2 + (0 if qm in (0, 1) else 1)
                    pending[0] = (ps_o, recip_f, hp, cc_buf, shard)

                for qm in (0, 2):
                    for b in range(B):
                        for h in range(HPC):
                            attn_unit(b, h, qm, cc_in1)
                flush_norm()
                with nc.named_scope("a2a1"):
                    nc.gpsimd.collective_compute(
                        "AllToAll", mybir.AluOpType.bypass,
                        ins=[cc_in1.opt()], outs=[cc_out1.opt()],
                        replica_groups=rg)
                for qm in (1, 3):
                    for b in range(B):
                        for h in range(HPC):
                            attn_unit(b, h, qm, cc_in2)
                flush_norm()
                with nc.named_scope("a2a2"):
                    nc.gpsimd.collective_compute(
                        "AllToAll", mybir.AluOpType.bypass,
                        ins=[cc_in2.opt()], outs=[cc_out2.opt()],
                        replica_groups=rg)

            # ---- phase C: output projection on owned tokens ----
            with (
                nc.named_scope("oproj"),
                tc.tile_pool(name="psC", bufs=3, space="PSUM") as psC,
                tc.tile_pool(name="spC", bufs=3) as sp,
                tc.tile_pool(name="scrC", bufs=3) as scr,
            ):
                for tt in range(TOK // 128):
                    cc_o = cc_out1 if tt < 4 else cc_out2
                    col = (tt % 4) * 128
                    a_sb = sp.tile([128, n_dt, 128], BF16, tag="a")
                    for dt in range(n_dt):
                        nc.sync.dma_start(
                            out=a_sb[:, dt, :],
                            in_=cc_o[dt, :, col:col + 128])
                    for et in range(D // 512):
                        ps = psC.tile([128, 512], F32, tag="ps_c")
                        for dt in range(n_dt):
                            nc.tensor.matmul(
                                ps[:, :], a_sb[:, dt, :],
                                wo_sb[:, dt, et * 512:(et + 1) * 512],
                                start=(dt == 0), stop=(dt == n_dt - 1))
                        o_sb = scr.tile([128, 512], F32, tag="o")
                        nc.vector.tensor_copy(o_sb, ps)
                        nc.sync.dma_start(
                            out=out[tt * 128:(tt + 1) * 128,
                                    et * 512:(et + 1) * 512],
                            in_=o_sb)
    nc.compile()
    return nc


def kernel(x, W_qkv, W_o):
    global _CACHED_NC, LAST_EXEC_NS
    if _CACHED_NC is None:
        _CACHED_NC = _build()
    nc = _CACHED_NC

    x = np.ascontiguousarray(x, dtype=np.float32)
    xT = np.ascontiguousarray(x.reshape(T, D).T)          # (1024, 8192)
    woT = np.ascontiguousarray(W_o.astype(np.float32).T).astype(ml_dtypes.bfloat16)
    in_maps = []
    for i in range(N_CORES):
        rows = []
        for blk in range(3):                               # Q, K, V blocks
            rows.append(W_qkv[blk * D + i * E: blk * D + (i + 1) * E, :])
        wqkvT = np.ascontiguousarray(
            np.concatenate(rows, axis=0).astype(np.float32).T)  # (1024, 384)
        in_maps.append({"xT": xT, "wqkvT": wqkvT, "woT": woT})

    res = run_bass_kernel_spmd(nc, in_maps, core_ids=list(range(N_CORES)),
                               trace=TRACE)
    LAST_EXEC_NS = res.exec_time_ns
    kernel.LAST_RES = res
    full = np.empty((T, D), dtype=np.float32)
    for i in range(N_CORES):
        full[i * TOK:(i + 1) * TOK, :] = res.results[i]["out"]
    return full.reshape(B, S, D)


# revision 19
# speedup vs baseline: 1.7833x; 1.0795x over previous
"""Distributed causal multi-head attention for TRN2 (8 NeuronCores).

Sharding: tensor-parallel over heads — core i computes heads {2i, 2i+1}
(128 of the 1024 hidden dims) for the whole (batch, seq) = (4, 2048).
All attention runs in transposed layouts (Q^T/K^T as [dh, t], scores as
[k, q]) so no on-chip transposes are needed except a cheap V^T -> V pass.
A single 8-rank AllToAll re-shards from head-parallel to token-parallel
before the output projection: core i ends up with the full 1024-dim
attention output for tokens [i*1024, (i+1)*1024) of the flattened
(8192,) token axis and computes that slice of out = attn @ W_o^T.

Matmuls run as float32r (single-pass fp32 on the PE, ~4x plain fp32).
"""

import sys

sys.path.insert(0, "/opt/trn_rl_repo")

import numpy as np
import ml_dtypes

import concourse.bass as bass
import concourse.tile as tile
from concourse import bacc, mybir
from concourse.bass_utils import run_bass_kernel_spmd
from concourse.masks import make_identity

F32 = mybir.dt.float32
F32R = mybir.dt.float32r
BF16 = mybir.dt.bfloat16

B, S, D = 4, 2048, 1024
N_HEAD, D_HEAD = 16, 64
T = B * S               # 8192 flattened tokens
N_CORES = 8
HPC = N_HEAD // N_CORES  # heads per core = 2
E = HPC * D_HEAD         # 128 local attn dims per core
TW = 512                 # token window for QKV phase
QM = 512                 # query macro-tile for attention
KT = 128                 # key tile
TOK = T // N_CORES       # 1024 tokens owned per core after A2A
SCALE = 1.0 / 8.0        # 1/sqrt(64)
NEG = -1e9

TRACE = False
LAST_EXEC_NS = None
_CACHED_NC = None


def _build():
    nc = bacc.Bacc("TRN2", target_bir_lowering=False, debug=False,
                   num_devices=N_CORES)
    xT = nc.dram_tensor("xT", [D, T], F32R, kind="ExternalInput").ap()
    wqkvT = nc.dram_tensor("wqkvT", [D, 3 * E], F32R, kind="ExternalInput").ap()
    woT = nc.dram_tensor("woT", [D, D], BF16, kind="ExternalInput").ap()
    out = nc.dram_tensor("out", [TOK, D], F32, kind="ExternalOutput").ap()
    cc_in1 = nc.dram_tensor("cc_in1", [N_CORES, E, TOK // 2], BF16).ap()
    cc_out1 = nc.dram_tensor("cc_out1", [N_CORES, E, TOK // 2], BF16).ap()
    cc_in2 = nc.dram_tensor("cc_in2", [N_CORES, E, TOK // 2], BF16).ap()
    cc_out2 = nc.dram_tensor("cc_out2", [N_CORES, E, TOK // 2], BF16).ap()
    rg = [list(range(N_CORES))]

    n_dt = D // 128          # 8 d-tiles (contraction for projections)
    n_tw = T // TW           # 16 token windows
    n_kt = T // KT           # 64 key tiles total

    with tile.TileContext(nc) as tc:
        with (
            tc.tile_pool(name="persist", bufs=1) as pp,
            tc.tile_pool(name="weights", bufs=1) as wp,
            tc.tile_pool(name="psA", bufs=2, space="PSUM") as psA,
            tc.tile_pool(name="psS", bufs=4, space="PSUM") as psS,
            tc.tile_pool(name="psO", bufs=2, space="PSUM") as psO,
            tc.tile_pool(name="spA", bufs=3) as spw,
            tc.tile_pool(name="scrA", bufs=2) as scrv,
            tc.tile_pool(name="scrBp", bufs=4) as scr,
            tc.tile_pool(name="scrB2", bufs=2) as scr2,
            tc.tile_pool(name="spC", bufs=3) as spc,
            tc.tile_pool(name="scrC", bufs=3) as scrc,
        ):
            # ---- persistent SBUF ----
            qt = pp.tile([128, HPC, T], BF16, tag="qt")   # zero-padded per head
            kt_sb = pp.tile([E, T], BF16, tag="kt")       # K^T [128, 8192]
            vb = pp.tile([128, n_kt, 2 * (D_HEAD + 1)], BF16, tag="vb")
            wk_sb = wp.tile([128, n_dt, 3 * E], F32R, tag="wk")
            for dt in range(n_dt):
                nc.sync.dma_start(out=wk_sb[:, dt, :],
                                  in_=wqkvT[dt * 128:(dt + 1) * 128, :])

            # 0/1 bf16 masks for the 4 diagonal k-tile offsets
            masks = []
            for j in range(4):
                m = wp.tile([KT, QM], BF16, tag=f"mask{j}")
                nc.gpsimd.memset(m, 1.0)
                nc.gpsimd.affine_select(
                    out=m, in_=m, compare_op=mybir.AluOpType.is_ge,
                    fill=0.0, base=-(j * KT),
                    pattern=[[1, QM]], channel_multiplier=-1,
                )
                masks.append(m)
            ident_f = wp.tile([128, 128], F32, tag="idf")
            make_identity(nc, ident_f)
            ident = wp.tile([128, 128], BF16, tag="idr")
            nc.vector.tensor_copy(ident, ident_f)
            ones_f = wp.tile([128, n_kt], F32, tag="ones")
            nc.vector.memset(ones_f, 1.0)
            wo_sb = wp.tile([128, n_dt, D], BF16, tag="wo")
            for dt in range(n_dt):
                nc.sync.dma_start(out=wo_sb[:, dt, :],
                                  in_=woT[dt * 128:(dt + 1) * 128, :])

            # zero qt pad rows; ones columns in vb — all written up front so
            # attention tiles never wait on later QKV windows
            nc.vector.memset(qt[D_HEAD:128, 0, :], 0.0)
            nc.vector.memset(qt[0:D_HEAD, 1, :], 0.0)
            for h in range(HPC):
                nc.vector.tensor_copy(
                    vb[:, :, h * (D_HEAD + 1) + D_HEAD:
                           h * (D_HEAD + 1) + D_HEAD + 1],
                    ones_f[:, :, None])

            # ---- emission helpers ----
            def qkv_window(tw):
                xw = spw.tile([128, n_dt, TW], F32R, tag="xw")
                for dt in range(n_dt):
                    nc.sync.dma_start(
                        out=xw[:, dt, :],
                        in_=xT[dt * 128:(dt + 1) * 128,
                               tw * TW:(tw + 1) * TW])
                for which in (0, 1):
                    ps = psA.tile([128, TW], F32, tag="a")
                    for dt in range(n_dt):
                        nc.tensor.matmul(
                            ps[0:E, :],
                            wk_sb[:, dt, which * E:(which + 1) * E],
                            xw[:, dt, :],
                            start=(dt == 0), stop=(dt == n_dt - 1))
                    if which == 1:
                        nc.vector.tensor_copy(
                            kt_sb[:, tw * TW:(tw + 1) * TW], ps[0:E, :])
                    else:
                        for h in range(HPC):
                            nc.vector.tensor_copy(
                                qt[h * D_HEAD:(h + 1) * D_HEAD, h,
                                   tw * TW:(tw + 1) * TW],
                                ps[h * D_HEAD:(h + 1) * D_HEAD, :])
                # V^T then transpose into natural layout
                ps = psA.tile([128, TW], F32, tag="a")
                for dt in range(n_dt):
                    nc.tensor.matmul(
                        ps[0:E, :], wk_sb[:, dt, 2 * E:3 * E],
                        xw[:, dt, :],
                        start=(dt == 0), stop=(dt == n_dt - 1))
                vt_sb = scrv.tile([E, TW], BF16, tag="vt")
                nc.vector.tensor_copy(vt_sb, ps[0:E, :])
                for sub in range(TW // 128):
                    k_idx = tw * (TW // 128) + sub
                    pst = psA.tile([128, 128], BF16, tag="a")
                    nc.tensor.transpose(
                        pst[:, 0:E], vt_sb[:, sub * 128:(sub + 1) * 128],
                        ident)
                    nc.vector.tensor_copy(
                        vb[:, k_idx, :].rearrange(
                            "p (h c) -> p h c", c=D_HEAD + 1)[:, :, 0:D_HEAD],
                        pst[:, 0:E].rearrange(
                            "p (h c) -> p h c", c=D_HEAD))

            pending = [None]

            def flush_norm():
                if pending[0] is None:
                    return
                ps_o_p, rec_p, hp_p, buf_p, shard_p = pending[0]
                pending[0] = None
                rb = scr2.tile([D_HEAD, QM], F32, tag="rb")
                nc.gpsimd.partition_broadcast(rb, rec_p)
                num_sb = scr2.tile([D_HEAD, QM], F32R, tag="num")
                nc.vector.tensor_copy(num_sb, ps_o_p[0:D_HEAD, :])
                attn_sb = scr2.tile([D_HEAD, QM], BF16, tag="attn")
                nc.vector.tensor_mul(attn_sb, num_sb, rb)
                nc.sync.dma_start(
                    out=buf_p[shard_p, hp_p:hp_p + D_HEAD, :],
                    in_=attn_sb)

            def attn_unit(b, h, qm, cc_buf):
                hp = h * D_HEAD
                q0 = b * S + qm * QM
                n_k = 4 * qm + 4
                ps_o = psO.tile([128, QM], F32, tag="ps_o")
                pts = [None] * n_k

                def emit_scores(k):
                    k0 = b * S + k * KT
                    ps_s = psS.tile([KT, QM], F32, tag="ps_s")
                    nc.tensor.matmul(
                        ps_s[:, :],
                        kt_sb[:, k0:k0 + KT],
                        qt[:, h, q0:q0 + QM],
                        start=True, stop=True)
                    pt = scr.tile([KT, QM], BF16, tag="pt")
                    if k >= 4 * qm:  # diagonal: exp then zero-mask
                        pe = scr.tile([KT, QM], BF16, tag="pe")
                        nc.scalar.activation(
                            out=pe, in_=ps_s,
                            func=mybir.ActivationFunctionType.Exp,
                            scale=SCALE)
                        nc.vector.tensor_mul(pt, pe, masks[k - 4 * qm])
                    else:
                        nc.scalar.activation(
                            out=pt, in_=ps_s,
                            func=mybir.ActivationFunctionType.Exp,
                            scale=SCALE)
                    pts[k] = pt

                LA = 3   # score/exp lookahead so PE never waits
                for k in range(min(LA, n_k)):
                    emit_scores(k)
                flush_norm()   # previous macro-tile's epilogue
                for k in range(n_k):
                    if k + LA < n_k:
                        emit_scores(k + LA)
                    nc.tensor.matmul(
                        ps_o[0:D_HEAD + 1, :],
                        vb[:, b * (S // KT) + k,
                           h * (D_HEAD + 1):(h + 1) * (D_HEAD + 1)],
                        pts[k],
                        start=(k == 0), stop=(k == n_k - 1))
                    pts[k] = None
                den_sb = scr2.tile([1, QM], F32, tag="den")
                nc.vector.tensor_copy(den_sb, ps_o[D_HEAD:D_HEAD + 1, :])
                recip_f = scr2.tile([1, QM], F32, tag="recipf")
                nc.vector.reciprocal_approx_fast(out=recip_f, in_=den_sb)
                shard = b * 2 + (0 if qm in (0, 1) else 1)
                pending[0] = (ps_o, recip_f, hp, cc_buf, shard)

            def oproj_tile(tt):
                cc_o = cc_out1 if tt < 4 else cc_out2
                col = (tt % 4) * 128
                a_sb = spc.tile([128, n_dt, 128], BF16, tag="a")
                for dt in range(n_dt):
                    nc.sync.dma_start(
                        out=a_sb[:, dt, :],
                        in_=cc_o[dt, :, col:col + 128])
                for et in range(D // 512):
                    ps = psA.tile([128, 512], F32, tag="a")
                    for dt in range(n_dt):
                        nc.tensor.matmul(
                            ps[:, :], a_sb[:, dt, :],
                            wo_sb[:, dt, et * 512:(et + 1) * 512],
                            start=(dt == 0), stop=(dt == n_dt - 1))
                    o_sb = scrc.tile([128, 512], F32, tag="o")
                    nc.vector.tensor_copy(o_sb, ps)
                    nc.sync.dma_start(
                        out=out[tt * 128:(tt + 1) * 128,
                                et * 512:(et + 1) * 512],
                        in_=o_sb)

            # ---- emission schedule: QKV(b+1) interleaves attention(b) ----
            for tw in range(4):
                qkv_window(tw)
            for b in range(B):
                units = [(b, h, qm) for qm in (0, 2) for h in range(HPC)]
                for i, (bb, h, qm) in enumerate(units):
                    attn_unit(bb, h, qm, cc_in1)
                    if b < B - 1:
                        qkv_window(4 * (b + 1) + i)
            flush_norm()
            with nc.named_scope("a2a1"):
                nc.gpsimd.collective_compute(
                    "AllToAll", mybir.AluOpType.bypass,
                    ins=[cc_in1.opt()], outs=[cc_out1.opt()],
                    replica_groups=rg)
            # pass 2 with O-proj half 1 spread between units
            p2units = [(b, h, qm) for b in range(B)
                       for qm in (1, 3) for h in range(HPC)]
            for i, (b, h, qm) in enumerate(p2units):
                attn_unit(b, h, qm, cc_in2)
                if i % 4 == 3 and i // 4 < 4:
                    oproj_tile(i // 4)
            flush_norm()
            with nc.named_scope("a2a2"):
                nc.gpsimd.collective_compute(
                    "AllToAll", mybir.AluOpType.bypass,
                    ins=[cc_in2.opt()], outs=[cc_out2.opt()],
                    replica_groups=rg)
            for tt in range(4, 8):
                oproj_tile(tt)
    nc.compile()
    return nc


def kernel(x, W_qkv, W_o):
    global _CACHED_NC, LAST_EXEC_NS
    if _CACHED_NC is None:
        _CACHED_NC = _build()
    nc = _CACHED_NC

    x = np.ascontiguousarray(x, dtype=np.float32)
    xT = np.ascontiguousarray(x.reshape(T, D).T)          # (1024, 8192)
    woT = np.ascontiguousarray(W_o.astype(np.float32).T).astype(ml_dtypes.bfloat16)
    in_maps = []
    for i in range(N_CORES):
        rows = []
        for blk in range(3):                               # Q, K, V blocks
            rows.append(W_qkv[blk * D + i * E: blk * D + (i + 1) * E, :])
        wqkvT = np.ascontiguousarray(
            np.concatenate(rows, axis=0).astype(np.float32).T)  # (1024, 384)
        in_maps.append({"xT": xT, "wqkvT": wqkvT, "woT": woT})

    res = run_bass_kernel_spmd(nc, in_maps, core_ids=list(range(N_CORES)),
                               trace=TRACE)
    LAST_EXEC_NS = res.exec_time_ns
    kernel.LAST_RES = res
    full = np.empty((T, D), dtype=np.float32)
    for i in range(N_CORES):
        full[i * TOK:(i + 1) * TOK, :] = res.results[i]["out"]
    return full.reshape(B, S, D)
